# revision 7
# baseline (speedup 1.0000x reference)
"""Trainium2 8-core kernel for nn_CAT_81269371175150 (GNN message passing).

Math (see reference):
  gcn(x)   = selu(A_gn @ (x @ W1^T))            for features and aug_features
  S        = softmax_K(gcn1 @ Wt^T)
  loss     = spectral(S, A) + cluster(S) + 0.5 * con(gcn1, gcn2)

Strategy (v3: pipelined chunked AllGather over the fp8 v2 pipeline):
  * Nodes sharded row-wise across 8 cores.  Phase A computes
    h1|h2 = [X|Xa] @ W1^T in three block passes (14/14/21 blocks); each
    pass's rows are AllGathered as soon as they are ready (3 chunked AGs
    on the cc stream), so the SpMM gather stream starts right after AG_0
    instead of after one monolithic AllGather (~120us earlier).
  * Edges are bucketed by (source table q, dest block b) and gathered
    q-major: all q=0 groups (sources in pass-0 rows of any core), then
    q=1, then q=2.  Per (q,b) group: dma_gather of fp8 512B combined
    [h1|h2] rows (4 SWDGE queues, negative-index padding skip) + one fp8
    one-hot matmul per 128-edge chunk (gn folded in, host-built).
  * PSUM cannot hold 49 open accumulators, so q=0/q=1 partial sums spill
    to a bf16 SBUF accumulator per dest block (the gathered table is fp8,
    so bf16 partial accumulation error is negligible); q=2 closes each
    block: selu (single [128,512] chain), con-loss partials, S softmax.
  * log-softmax stats have no max subtraction (selu outputs are small
    enough that fp32/bf16 exp cannot overflow): exp column-sums
    accumulate into one persistent PSUM bank via tiny per-block matmuls,
    which kills the old gcn1T persistence + segmented-stats tail.
  * Host finishes the tiny reductions: trace(S^T A S), nl, cluster sizes,
    logZ across cores, final scalar.
"""

import math
import numpy as np
import ml_dtypes

import concourse.bacc as bacc
import concourse.mybir as mybir
import concourse.tile as tile
from concourse import bass_utils
from concourse.masks import make_identity

P = 128
NC = 8
NQ = 3                    # AllGather chunks / source tables
PASS_BLOCKS = (14, 14, 21)  # phase-A pass sizes in blocks (sum = NB = 49)
RING = 8                  # gather tile ring depth
OH_BATCH = 4              # one-hot load batching (groups per dma)

# full-size problem constants
FULL = dict(N=50000, F=500, D=256, K=16)

SELU_L = 1.0507009873554805
SELU_A = 1.6732632423543772
SELU_LA = SELU_L * SELU_A
LN_SELU_LA = math.log(SELU_LA)

CLUSTER_REG = 1.0
CON_REG = 0.5

bf16 = mybir.dt.bfloat16
fp8 = mybir.dt.float8e4
f32 = mybir.dt.float32
i16 = mybir.dt.int16
i32 = mybir.dt.int32


def cdiv(a, b):
    return -(-a // b)


# --------------------------------------------------------------------------
# host-side preprocessing
# --------------------------------------------------------------------------

def prep(features, aug_features, graph_row, graph_col, gn_vals, W1, Wt, cfg):
    N, F, D, K = cfg["N"], cfg["F"], cfg["D"], cfg["K"]
    NSH = N // NC
    NB = cdiv(NSH, P)
    assert sum(PASS_BLOCKS) == NB

    # pass row offsets within a shard, and per-pass row counts
    PB = np.concatenate([[0], np.cumsum(PASS_BLOCKS)])        # block bounds
    RO = np.minimum(PB * P, NSH)                              # row bounds
    rows_q = [int(RO[q + 1] - RO[q]) for q in range(NQ)]

    row = np.asarray(graph_row).astype(np.int64)
    col = np.asarray(graph_col).astype(np.int64)
    gn = np.asarray(gn_vals).astype(np.float64)

    core = row // NSH
    per_core = []
    cnts = np.zeros((NC, NQ, NB), dtype=np.int64)
    for c in range(NC):
        m = core == c
        r = row[m] - c * NSH
        cl = col[m]
        g = gn[m]
        b = r // P
        src_core = cl // NSH
        off = cl % NSH
        q = np.digitize(off, RO[1:NQ])        # 0..NQ-1 by source pass
        loc = src_core * np.array(rows_q)[q] + (off - RO[q])
        order = np.lexsort((cl, b, q))
        r, cl, g, b, q, loc = (r[order], cl[order], g[order], b[order],
                               q[order], loc[order])
        key = q * NB + b
        cnt = np.bincount(key, minlength=NQ * NB).reshape(NQ, NB)
        cnts[c] = cnt
        per_core.append((r, g, b, q, loc, key))

    CQB = np.ceil(cnts.max(axis=0) / P).astype(np.int64)      # [NQ, NB]
    strm_base = np.zeros((NQ, NB), dtype=np.int64)
    for q in range(NQ):
        strm_base[q] = np.concatenate([[0], np.cumsum(CQB[q])[:-1]])
    Lq = [int(CQB[q].sum()) * P for q in range(NQ)]
    nch_flat = CQB.reshape(-1)                                # q-major
    ohbase = np.concatenate([[0], np.cumsum(nch_flat)[:-1]])
    NCHT = int(nch_flat.sum())
    emitted = [(q, b) for q in range(NQ) for b in range(NB)
               if CQB[q][b] > 0]
    emit_rank = {g: i for i, g in enumerate(emitted)}

    X = np.asarray(features)[0]
    Xa = np.asarray(aug_features)[0]
    XT = np.ascontiguousarray(X.T).astype(ml_dtypes.float8_e4m3)   # [F, N]
    XTa = np.ascontiguousarray(Xa.T).astype(ml_dtypes.float8_e4m3)
    W1T = np.ascontiguousarray(np.asarray(W1).T).astype(ml_dtypes.bfloat16)
    WtT = np.ascontiguousarray(np.asarray(Wt).T).astype(ml_dtypes.bfloat16)

    def wrap_idx(a):
        # [L] -> [128, L/16]: element i at [i%16, i//16], replicated x8
        L = a.shape[0]
        w = a.reshape(L // 16, 16).T
        return np.ascontiguousarray(np.tile(w, (8, 1)))

    in_maps = []
    for c in range(NC):
        r, g, b, q, loc, key = per_core[c]
        cnt = cnts[c]
        run_start = np.zeros(NQ * NB, dtype=np.int64)
        flat = cnt.reshape(-1)
        run_start[1:] = np.cumsum(flat)[:-1]
        rank = np.arange(len(r)) - run_start[key]
        lane = rank % P
        j = rank // P

        idx_streams = []
        gcnt = np.zeros(NQ * NB, dtype=np.int32)
        for qq in range(NQ):
            arr = np.full(Lq[qq], -1, dtype=np.int16)
            m = q == qq
            offp = (strm_base[qq][b[m]] + j[m]) * P + lane[m]
            arr[offp] = loc[m].astype(np.int16)
            for bb in range(NB):
                n = CQB[qq][bb]
                if n == 0:
                    continue
                base = strm_base[qq][bb] * P
                cv = int(cnt[qq][bb])
                if emit_rank.get((qq, bb), 0) < RING:
                    eff = n * P          # first ring pass: gather everything
                else:
                    eff = min(max(cdiv(cv, 16) * 16, P), n * P)
                arr[base + cv:base + eff] = 0
                gcnt[qq * NB + bb] = eff
            idx_streams.append(wrap_idx(arr))

        oh = np.zeros((P, NCHT, P), dtype=ml_dtypes.float8_e4m3)
        ohcol = ohbase[key] + j
        dest = r - b * P
        oh[lane, ohcol, dest] = g.astype(ml_dtypes.float8_e4m3)

        in_maps.append({
            "xt": np.ascontiguousarray(XT[:, c * NSH:(c + 1) * NSH]),
            "xta": np.ascontiguousarray(XTa[:, c * NSH:(c + 1) * NSH]),
            "w1t": W1T,
            "wtt": WtT,
            "oh": oh,
            "idx0": idx_streams[0],
            "idx1": idx_streams[1],
            "idx2": idx_streams[2],
            "gcnt": gcnt.reshape(1, -1),
        })

    meta = dict(
        N=N, F=F, D=D, K=K, NSH=NSH, NB=NB, DT=D // P,
        rows_q=tuple(rows_q), PB=tuple(int(x) for x in PB),
        CQB=tuple(map(tuple, CQB.tolist())), NCHT=NCHT,
        strm_base=tuple(map(tuple, strm_base.tolist())),
        Lq=tuple(Lq), ohbase=tuple(int(x) for x in ohbase),
        FT=cdiv(F, P),
    )
    return in_maps, meta


# --------------------------------------------------------------------------
# device program
# --------------------------------------------------------------------------

def build(meta, debug=False):
    N, F, D, K = meta["N"], meta["F"], meta["D"], meta["K"]
    NSH, NB, DT = meta["NSH"], meta["NB"], meta["DT"]
    rows_q = meta["rows_q"]
    PB = meta["PB"]
    CQB = meta["CQB"]
    NCHT = meta["NCHT"]
    strm_base = meta["strm_base"]
    Lq = meta["Lq"]
    ohbase = meta["ohbase"]
    FT = meta["FT"]
    W2 = 2 * D                  # combined table row elems (bf16)
    VLEN = 2 * D

    nc = bacc.Bacc("TRN2", target_bir_lowering=False, debug=debug,
                   num_devices=NC, num_swdge_queues=4,
                   dynamic_dma_scratch_size=65536)

    xt = nc.dram_tensor("xt", [F, NSH], fp8, kind="ExternalInput")
    xta = nc.dram_tensor("xta", [F, NSH], fp8, kind="ExternalInput")
    w1t = nc.dram_tensor("w1t", [F, D], bf16, kind="ExternalInput")
    wtt = nc.dram_tensor("wtt", [D, K], bf16, kind="ExternalInput")
    oh = nc.dram_tensor("oh", [P, NCHT, P], fp8, kind="ExternalInput")
    idx_d = [nc.dram_tensor(f"idx{q}", [P, Lq[q] // 16], i16,
                            kind="ExternalInput")
             for q in range(NQ)]
    gcnt_d = nc.dram_tensor("gcnt", [1, NQ * NB], i32, kind="ExternalInput")

    stats_v_d = nc.dram_tensor("stats_v", [1, VLEN], f32,
                               kind="ExternalOutput")
    stats_e_d = nc.dram_tensor("stats_e", [P, DT], f32, kind="ExternalOutput")
    s_out_d = nc.dram_tensor("s_out", [NB * P, K], f32, kind="ExternalOutput")

    max_grp = max((CQB[q][b] for b in range(NB) for q in range(NQ)),
                  default=1)
    gorder = [(q, b) for q in range(NQ) for b in range(NB) if CQB[q][b] > 0]

    # one-hot load batches in consumption (= emission) order
    batches = []          # (cstart, cend); group -> (batch idx, col offset)
    group_batch = {}
    for i in range(0, len(gorder), OH_BATCH):
        gs = gorder[i:i + OH_BATCH]
        cstart = ohbase[gs[0][0] * NB + gs[0][1]]
        cend = ohbase[gs[-1][0] * NB + gs[-1][1]] + CQB[gs[-1][0]][gs[-1][1]]
        bi = len(batches)
        batches.append((cstart, cend))
        for (q, b) in gs:
            group_batch[(q, b)] = (bi, ohbase[q * NB + b] - cstart)
    max_batch_nch = max((ce - cs for cs, ce in batches), default=1)

    with tile.TileContext(nc) as tc:
        with (
            tc.tile_pool(name="big", bufs=8) as bigp,
            tc.tile_pool(name="gtp", bufs=RING) as gtp,
            tc.tile_pool(name="ohp", bufs=3) as ohp,
            tc.tile_pool(name="persist", bufs=1) as persist,
            tc.tile_pool(name="stage", bufs=3) as stagep,
            tc.tile_pool(name="tmp", bufs=2) as tmpp,
            tc.tile_pool(name="small", bufs=4) as smallp,
            tc.tile_pool(name="svp", bufs=1) as svp,
            tc.tile_pool(name="pa", bufs=4, space="PSUM") as pa,
            tc.tile_pool(name="pb", bufs=3, space="PSUM") as pb,
            tc.tile_pool(name="red", bufs=1, space="PSUM") as redp,
            tc.tile_pool(name="dram", bufs=1, space="DRAM") as dramp,
        ):
            # ---- constants / resident tensors
            ident = persist.tile([P, P], f32)
            make_identity(nc, ident[:])
            w1t_t = persist.tile([P, FT, D], bf16)
            for t in range(FT):
                fr = min(P, F - t * P)
                nc.sync.dma_start(w1t_t[:fr, t, :], w1t[t * P:t * P + fr, :])
            wtt_t = persist.tile([P, DT, K], bf16)
            for t in range(DT):
                nc.sync.dma_start(wtt_t[:, t, :], wtt[t * P:(t + 1) * P, :])
            idx_t = []
            for q in range(NQ):
                it = persist.tile([P, Lq[q] // 16], i16, tag=f"idx{q}")
                nc.sync.dma_start(it[:], idx_d[q][:])
                idx_t.append(it)
            gcnt_t = persist.tile([1, NQ * NB], i32, tag="gcnt")
            nc.sync.dma_start(gcnt_t[:], gcnt_d[:])

            ln_la = persist.tile([P, 1], f32, tag="lnla")
            nc.vector.memset(ln_la[:], LN_SELU_LA)
            la_c = persist.tile([P, 1], f32, tag="lac")
            nc.vector.memset(la_c[:], SELU_LA)
            ones_b = persist.tile([P, 1], bf16, tag="onesb")
            nc.vector.memset(ones_b[:], 1.0)
            ones_f = persist.tile([P, 1], f32, tag="onesf")
            nc.vector.memset(ones_f[:], 1.0)
            # masked ones: 1.0 for partitions < last-block rows, else 0
            # (partition-offset memset is rejected by the verifier, so
            # build it as a row-slice reduction of the identity matrix)
            last_rows = NSH - (NB - 1) * P
            ones_mf = persist.tile([P, 1], f32, tag="onesmf")
            nc.vector.reduce_sum(ones_mf[:], ident[:, 0:last_rows],
                                 axis=mybir.AxisListType.X)
            ones_m = persist.tile([P, 1], bf16, tag="onesm")
            nc.vector.tensor_copy(ones_m[:], ones_mf[:])

            accs = persist.tile([P, VLEN], f32, tag="accs")
            nc.vector.memset(accs[:], 0.0)
            acc_blocks = persist.tile([P, NB, W2], bf16, tag="accb")

            red = redp.tile([P, DT], f32, tag="red")

            cc_in = [dramp.tile([rows_q[q], W2], fp8, name=f"cc_in{q}")
                     for q in range(NQ)]
            cc_out = [dramp.tile([NC * rows_q[q], W2], fp8,
                                 addr_space="Shared", name=f"cc_out{q}")
                      for q in range(NQ)]

            # ================= phase A: h1|h2 = [X|Xa] @ W1^T =============
            # one pass per AG chunk; each pass's rows AllGather as soon as
            # the pass's writes complete, so SpMM gathers on table 0 can
            # overlap the later AG chunks.
            for q in range(NQ):
                b_lo, b_hi = PB[q], PB[q + 1]
                c0 = b_lo * P
                c1 = min(b_hi * P, NSH)
                W = c1 - c0
                xt_tiles = []
                for which, src in enumerate((xt, xta)):
                    tl = []
                    for t in range(FT):
                        fr = min(P, F - t * P)
                        xx = bigp.tile([P, W], fp8, tag="big",
                                       name=f"xx{q}_{which}_{t}")
                        eng = nc.sync if (which * FT + t) % 2 == 0 else nc.scalar
                        eng.dma_start(xx[:fr, :], src[t * P:t * P + fr, c0:c1])
                        tl.append(xx)
                    xt_tiles.append(tl)
                for b in range(b_lo, b_hi):
                    rows = min(P, NSH - b * P)
                    off = b * P - c0
                    ptw = pb.tile([P, W2], f32, space="PSUM", tag="pb")
                    for which in range(2):
                        for t in range(FT):
                            fr = min(P, F - t * P)
                            nc.tensor.matmul(
                                ptw[:rows, which * D:(which + 1) * D],
                                lhsT=xt_tiles[which][t][:fr, off:off + rows],
                                rhs=w1t_t[:fr, t, :],
                                start=(t == 0), stop=(t == FT - 1),
                            )
                    st = stagep.tile([P, W2], fp8, tag="stage")
                    nc.vector.tensor_copy(st[:rows, :], ptw[:rows, :])
                    nc.scalar.dma_start(cc_in[q][b * P - c0:b * P - c0 + rows, :],
                                        st[:rows, :])
                nc.gpsimd.collective_compute(
                    "AllGather", mybir.AluOpType.bypass,
                    replica_groups=[list(range(NC))],
                    ins=[cc_in[q][:]], outs=[cc_out[q][:]],
                )

            # ================= gather stream (q-major emission) ===========
            gtile = {}
            prev_inst = None
            gq = 0
            cnt_reg = nc.gpsimd.alloc_register("gcnt_reg")
            for er, (q, b) in enumerate(gorder):
                sc = strm_base[q][b]
                n = CQB[q][b]
                gt = gtp.tile([P, max_grp, W2], fp8, tag="gt",
                              name=f"gt_{q}_{b}")
                if er < RING and n < max_grp:
                    # first ring pass leaves slices >= n uninitialized;
                    # zero them so later pad lanes never read NaN bits
                    nc.vector.memset(gt[:, n:max_grp, :], 0.0)
                ld = nc.gpsimd.reg_load(
                    cnt_reg, gcnt_t[0:1, q * NB + b:q * NB + b + 1])
                if prev_inst is not None:
                    tile.add_dep_helper(ld.ins, prev_inst, sync=False,
                                        reason="gather issue order")
                gi = nc.gpsimd.dma_gather(
                    gt[:, 0:n, :], cc_out[q][:],
                    idx_t[q][:, sc * 8:(sc + n) * 8],
                    num_idxs=n * P, num_idxs_reg=cnt_reg, elem_size=W2,
                    single_packet=False,
                    queue_num=gq % 4,
                )
                gq += 1
                tile.add_dep_helper(gi.ins, ld.ins, sync=False,
                                    reason="count reg load order")
                prev_inst = gi.ins
                gtile[(q, b)] = gt

            # ================= SpMM consumption + epilogues ===============
            oh_tiles = {}

            def get_oh(bi):
                if bi not in oh_tiles:
                    cs, ce = batches[bi]
                    oht = ohp.tile([P, max_batch_nch, P], fp8, tag="oh",
                                   name=f"oh_{bi}")
                    nc.scalar.dma_start(oht[:, 0:ce - cs, :],
                                        oh[:, cs:ce, :])
                    oh_tiles[bi] = oht
                return oh_tiles[bi]

            # issue the first few oh batch loads up front
            for bi in range(min(2, len(batches))):
                get_oh(bi)

            for q in range(NQ):
                for b in range(NB):
                    n = CQB[q][b]
                    rows = min(P, NSH - b * P)
                    accb = acc_blocks[:, b, :]
                    if n == 0:
                        if q == 0:
                            nc.vector.memset(accb, 0.0)
                        if q < NQ - 1:
                            continue
                    pt = None
                    if n > 0:
                        bi, coff = group_batch[(q, b)]
                        oht = get_oh(bi)
                        if bi + 1 < len(batches):
                            get_oh(bi + 1)
                        gt = gtile[(q, b)]
                        pt = pa.tile([P, W2], f32, space="PSUM", tag="pa")
                        for j in range(n):
                            nc.tensor.matmul(
                                pt[:], lhsT=oht[:, coff + j, :],
                                rhs=gt[:, j, :],
                                start=(j == 0), stop=(j == n - 1))
                    if q == 0:
                        nc.vector.tensor_copy(accb, pt[:])
                        continue
                    if q == 1:
                        nc.vector.tensor_tensor(accb, accb, pt[:],
                                                mybir.AluOpType.add)
                        continue

                    # ---- q == NQ-1: close the block ----
                    tot = tmpp.tile([P, W2], f32, tag="tot")
                    if pt is not None:
                        nc.vector.tensor_tensor(tot[:], pt[:], accb,
                                                mybir.AluOpType.add)
                    else:
                        nc.vector.tensor_copy(tot[:], accb)

                    # selu over the combined [128, 512] row:
                    # e2 = la*exp(x); e3 = relu(la - e2); r = relu(l*x)
                    # selu = r - e3
                    e2 = tmpp.tile([P, W2], f32, tag="e2")
                    nc.scalar.activation(e2[:], tot[:],
                                         mybir.ActivationFunctionType.Exp,
                                         bias=ln_la[:])
                    e3 = tmpp.tile([P, W2], f32, tag="e3")
                    nc.scalar.activation(e3[:], e2[:],
                                         mybir.ActivationFunctionType.Relu,
                                         bias=la_c[:], scale=-1.0)
                    slu = tmpp.tile([P, W2], f32, tag="slu")
                    nc.scalar.activation(slu[:], tot[:],
                                         mybir.ActivationFunctionType.Relu,
                                         scale=SELU_L)
                    nc.vector.tensor_tensor(slu[:], slu[:], e3[:],
                                            mybir.AluOpType.subtract)
                    g1 = slu[:, 0:D]
                    aug = slu[:, D:W2]

                    # con-loss partials (gpsimd to keep DVE off the
                    # critical path in the tail phase)
                    nc.gpsimd.tensor_tensor(accs[:, 0:D], accs[:, 0:D], aug,
                                            mybir.AluOpType.add)
                    pr = tmpp.tile([P, D], f32, tag="pr")
                    nc.gpsimd.tensor_tensor(pr[:], aug, g1,
                                            mybir.AluOpType.mult)
                    nc.gpsimd.tensor_tensor(accs[:, D:W2], accs[:, D:W2],
                                            pr[:], mybir.AluOpType.add)

                    # log-softmax stats: exp column-sums into persistent
                    # PSUM (no max subtraction needed; |g1| <~ 25)
                    eg = tmpp.tile([P, D], bf16, tag="eg")
                    nc.scalar.activation(eg[:], g1,
                                         mybir.ActivationFunctionType.Exp)
                    rvec = ones_m if rows < P else ones_b
                    for t in range(DT):
                        nc.tensor.matmul(red[:, t:t + 1],
                                         lhsT=eg[:, t * P:(t + 1) * P],
                                         rhs=rvec[:],
                                         start=(b == 0), stop=(b == NB - 1))

                    # S = softmax_K(g1 @ Wt^T)
                    g1T = tmpp.tile([P, D], bf16, tag="g1T")
                    for t in range(DT):
                        ptr = pb.tile([P, P], f32, space="PSUM", tag="pb")
                        nc.tensor.transpose(ptr[:], g1[:, t * P:(t + 1) * P],
                                            ident[:])
                        nc.vector.tensor_copy(g1T[:, t * P:(t + 1) * P],
                                              ptr[:])
                    pl = pb.tile([P, K], f32, space="PSUM", tag="pb")
                    for t in range(DT):
                        nc.tensor.matmul(pl[:],
                                         lhsT=g1T[:, t * P:(t + 1) * P],
                                         rhs=wtt_t[:, t, :],
                                         start=(t == 0), stop=(t == DT - 1))
                    nmx = smallp.tile([P, 1], f32, tag="nmx")
                    nc.vector.reduce_max(nmx[:], pl[:],
                                         axis=mybir.AxisListType.X,
                                         negate=True)
                    ex = smallp.tile([P, K], f32, tag="ex")
                    sm = smallp.tile([P, 1], f32, tag="sm")
                    nc.scalar.activation(ex[:], pl[:],
                                         mybir.ActivationFunctionType.Exp,
                                         bias=nmx[:], accum_out=sm[:])
                    rc = smallp.tile([P, 1], f32, tag="rc")
                    nc.vector.reciprocal(rc[:], sm[:])
                    sb = stagep.tile([P, K], f32, tag="sstage")
                    nc.scalar.mul(sb[:], ex[:], rc[:])
                    nc.sync.dma_start(s_out_d[b * P:(b + 1) * P, :], sb[:])

            # ---- final tiny reductions out
            pv = pa.tile([P, VLEN], f32, space="PSUM", tag="pa")
            nc.tensor.matmul(pv[0:1, :], lhsT=ones_f[:], rhs=accs[:],
                             start=True, stop=True)
            sv = svp.tile([1, VLEN], f32, tag="sv")
            nc.vector.tensor_copy(sv[:], pv[0:1, :])
            nc.sync.dma_start(stats_v_d[:], sv[:])
            se = svp.tile([P, DT], f32, tag="se")
            nc.vector.tensor_copy(se[:], red[:])
            nc.sync.dma_start(stats_e_d[:], se[:])

    nc.compile()
    return nc


# --------------------------------------------------------------------------
# host-side combine of per-core partials
# --------------------------------------------------------------------------

def combine(results, cfg, graph_row, graph_col, graph_vals):
    N, D, K = cfg["N"], cfg["D"], cfg["K"]
    NSH = N // NC
    DT = D // P
    E = float(graph_row.shape[0])

    colsum_aug = np.zeros(D)
    dot = 0.0
    expsum = np.zeros(D)
    S_full = np.zeros((N, K))
    for c in range(NC):
        sv = np.asarray(results[c]["stats_v"], dtype=np.float64).reshape(-1)
        colsum_aug += sv[0:D]
        dot += sv[D:2 * D].sum()
        se = np.asarray(results[c]["stats_e"], dtype=np.float64)  # [P, DT]
        expsum += se.T.reshape(-1)          # d = t*128 + r
        S_full[c * NSH:(c + 1) * NSH] = \
            np.asarray(results[c]["s_out"], dtype=np.float64)[:NSH]
    logZ = np.log(expsum)

    row = np.asarray(graph_row).astype(np.int64)
    col = np.asarray(graph_col).astype(np.int64)
    av = np.asarray(graph_vals).astype(np.float64)
    deg = np.bincount(col, weights=av, minlength=N).astype(np.float64)

    trace_gp = np.einsum('e,ek,ek->', av, S_full[row], S_full[col])
    nl = S_full.T @ deg
    clsz = S_full.sum(axis=0)

    spectral = -(trace_gp - (nl ** 2).sum() / (2.0 * E)) / (2.0 * E)
    cluster = (np.linalg.norm(clsz) / N * math.sqrt(K) - 1.0) * CLUSTER_REG
    con = -(dot - (logZ * colsum_aug).sum()) / D
    return spectral + cluster + CON_REG * con


# --------------------------------------------------------------------------
# entry point
# --------------------------------------------------------------------------

_BUILD_CACHE = {}


def kernel(features, aug_features, graph_row, graph_col, graph_vals, gn_vals,
           lbl, dense_graph, W1, b1, Wt, bt, _cfg=None, _trace=False):
    cfg = _cfg or FULL
    in_maps, meta = prep(features, aug_features, graph_row, graph_col,
                         gn_vals, W1, Wt, cfg)
    key = tuple(sorted((k, str(v)) for k, v in meta.items()))
    if key not in _BUILD_CACHE:
        _BUILD_CACHE[key] = build(meta)
    nc = _BUILD_CACHE[key]
    res = bass_utils.run_bass_kernel_spmd(nc, in_maps, core_ids=list(range(NC)),
                                          trace=_trace)
    loss = combine(res.results, cfg, graph_row, graph_col, graph_vals)
    out = np.array(loss, dtype=np.float32)
    if _trace:
        return out, res
    return out


# revision 22
# speedup vs baseline: 1.4126x; 1.4126x over previous
"""Trainium2 8-core kernel for nn_CAT_81269371175150 (GNN message passing).

Math (see reference):
  gcn(x)   = selu(A_gn @ (x @ W1^T))            for features and aug_features
  S        = softmax_K(gcn1 @ Wt^T)
  loss     = spectral(S, A) + cluster(S) + 0.5 * con(gcn1, gcn2)

Strategy (v4: two-chunk pipelined AllGather over the fp8 v2 pipeline):
  * Nodes sharded row-wise across 8 cores.  Phase A computes
    h1|h2 = [X|Xa] @ W1^T in two block passes (17/32 blocks); each
    pass's rows are AllGathered as soon as they are ready, so the SpMM
    gather stream starts right after AG_0 (~115us earlier than a
    monolithic AG).  The second chunk is deliberately LARGE: the phase
    that closes blocks is gather-rich, so the per-block epilogues hide
    under gather time instead of piling up in a short tail.
  * Edges are bucketed by (source table q, dest block b) and gathered
    q-major.  Per (q,b) group: dma_gather of fp8 512B combined [h1|h2]
    rows (4 SWDGE queues, negative-index padding skip) + one fp8
    one-hot matmul per 128-edge chunk (gn folded in, host-built).
    Nothing but gather dispatch runs on gpsimd: its sequencer executes
    in order, so any epilogue op there would stall later desc-gens.
  * PSUM cannot hold 49 open accumulators, so q=0 partial sums spill to
    a bf16 SBUF accumulator per dest block (the gathered table is fp8,
    so bf16 partial accumulation error is negligible); q=1 closes each
    block: selu (single [128,512] chain), con-loss partials, S softmax.
  * log-softmax stats have no max subtraction (selu outputs are small
    enough that fp32/bf16 exp cannot overflow): exp column-sums
    accumulate into one persistent PSUM bank via tiny per-block matmuls,
    which kills the old gcn1T persistence + segmented-stats tail.
  * Host finishes the tiny reductions: trace(S^T A S), nl, cluster sizes,
    logZ across cores, final scalar.
"""

import math
import numpy as np
import ml_dtypes

import concourse.bacc as bacc
import concourse.mybir as mybir
import concourse.tile as tile
from concourse import bass_utils
from concourse.masks import make_identity

P = 128
NC = 8
NQ = 2                    # AllGather chunks / source tables
PASS_BLOCKS = (17, 32)    # phase-A pass sizes in blocks (sum = NB = 49);
                          # pass 1 is capped by int16 gather indices:
                          # 8 cores * 4074 rows = 32592 < 32768
RING = 5                  # gather tile ring depth
OH_BATCH = 2              # one-hot load batching (groups per dma)

# full-size problem constants
FULL = dict(N=50000, F=500, D=256, K=16)

SELU_L = 1.0507009873554805
SELU_A = 1.6732632423543772
SELU_LA = SELU_L * SELU_A
LN_SELU_LA = math.log(SELU_LA)

CLUSTER_REG = 1.0
CON_REG = 0.5

bf16 = mybir.dt.bfloat16
fp8 = mybir.dt.float8e4
f32 = mybir.dt.float32
i16 = mybir.dt.int16
i32 = mybir.dt.int32


def cdiv(a, b):
    return -(-a // b)


# --------------------------------------------------------------------------
# host-side preprocessing
# --------------------------------------------------------------------------

def prep(features, aug_features, graph_row, graph_col, gn_vals, W1, Wt, cfg):
    N, F, D, K = cfg["N"], cfg["F"], cfg["D"], cfg["K"]
    NSH = N // NC
    NB = cdiv(NSH, P)
    assert sum(PASS_BLOCKS) == NB

    # pass row offsets within a shard, and per-pass row counts
    PB = np.concatenate([[0], np.cumsum(PASS_BLOCKS)])        # block bounds
    RO = np.minimum(PB * P, NSH)                              # row bounds
    rows_q = [int(RO[q + 1] - RO[q]) for q in range(NQ)]

    row = np.asarray(graph_row).astype(np.int64)
    col = np.asarray(graph_col).astype(np.int64)
    gn = np.asarray(gn_vals).astype(np.float64)

    core = row // NSH
    per_core = []
    cnts = np.zeros((NC, NQ, NB), dtype=np.int64)
    for c in range(NC):
        m = core == c
        r = row[m] - c * NSH
        cl = col[m]
        g = gn[m]
        b = r // P
        src_core = cl // NSH
        off = cl % NSH
        q = np.digitize(off, RO[1:NQ])        # 0..NQ-1 by source pass
        loc = src_core * np.array(rows_q)[q] + (off - RO[q])
        order = np.lexsort((cl, b, q))
        r, cl, g, b, q, loc = (r[order], cl[order], g[order], b[order],
                               q[order], loc[order])
        key = q * NB + b
        cnt = np.bincount(key, minlength=NQ * NB).reshape(NQ, NB)
        cnts[c] = cnt
        per_core.append((r, g, b, q, loc, key))

    CQB = np.ceil(cnts.max(axis=0) / P).astype(np.int64)      # [NQ, NB]
    strm_base = np.zeros((NQ, NB), dtype=np.int64)
    for q in range(NQ):
        strm_base[q] = np.concatenate([[0], np.cumsum(CQB[q])[:-1]])
    Lq = [int(CQB[q].sum()) * P for q in range(NQ)]
    nch_flat = CQB.reshape(-1)                                # q-major
    ohbase = np.concatenate([[0], np.cumsum(nch_flat)[:-1]])
    NCHT = int(nch_flat.sum())
    emitted = [(q, b) for q in range(NQ) for b in range(NB)
               if CQB[q][b] > 0]
    emit_rank = {g: i for i, g in enumerate(emitted)}

    X = np.asarray(features)[0]
    Xa = np.asarray(aug_features)[0]
    XT = np.ascontiguousarray(X.T).astype(ml_dtypes.float8_e4m3)   # [F, N]
    XTa = np.ascontiguousarray(Xa.T).astype(ml_dtypes.float8_e4m3)
    W1T = np.ascontiguousarray(np.asarray(W1).T).astype(ml_dtypes.bfloat16)
    WtT = np.ascontiguousarray(np.asarray(Wt).T).astype(ml_dtypes.bfloat16)

    def wrap_idx(a):
        # [L] -> [128, L/16]: element i at [i%16, i//16], replicated x8
        L = a.shape[0]
        w = a.reshape(L // 16, 16).T
        return np.ascontiguousarray(np.tile(w, (8, 1)))

    in_maps = []
    for c in range(NC):
        r, g, b, q, loc, key = per_core[c]
        cnt = cnts[c]
        run_start = np.zeros(NQ * NB, dtype=np.int64)
        flat = cnt.reshape(-1)
        run_start[1:] = np.cumsum(flat)[:-1]
        rank = np.arange(len(r)) - run_start[key]
        lane = rank % P
        j = rank // P

        idx_streams = []
        gcnt = np.zeros(NQ * NB, dtype=np.int32)
        for qq in range(NQ):
            arr = np.full(Lq[qq], -1, dtype=np.int16)
            m = q == qq
            offp = (strm_base[qq][b[m]] + j[m]) * P + lane[m]
            arr[offp] = loc[m].astype(np.int16)
            for bb in range(NB):
                n = CQB[qq][bb]
                if n == 0:
                    continue
                base = strm_base[qq][bb] * P
                cv = int(cnt[qq][bb])
                if emit_rank.get((qq, bb), 0) < RING:
                    eff = n * P          # first ring pass: gather everything
                else:
                    eff = min(max(cdiv(cv, 16) * 16, P), n * P)
                arr[base + cv:base + eff] = 0
                gcnt[qq * NB + bb] = eff
            idx_streams.append(wrap_idx(arr))

        oh = np.zeros((P, NCHT, P), dtype=ml_dtypes.float8_e4m3)
        ohcol = ohbase[key] + j
        dest = r - b * P
        oh[lane, ohcol, dest] = g.astype(ml_dtypes.float8_e4m3)

        im = {
            "xt": np.ascontiguousarray(XT[:, c * NSH:(c + 1) * NSH]),
            "xta": np.ascontiguousarray(XTa[:, c * NSH:(c + 1) * NSH]),
            "w1t": W1T,
            "wtt": WtT,
            "oh": oh,
            "gcnt": gcnt.reshape(1, -1),
        }
        for q in range(NQ):
            im[f"idx{q}"] = idx_streams[q]
        in_maps.append(im)

    meta = dict(
        N=N, F=F, D=D, K=K, NSH=NSH, NB=NB, DT=D // P,
        rows_q=tuple(rows_q), PB=tuple(int(x) for x in PB),
        CQB=tuple(map(tuple, CQB.tolist())), NCHT=NCHT,
        strm_base=tuple(map(tuple, strm_base.tolist())),
        Lq=tuple(Lq), ohbase=tuple(int(x) for x in ohbase),
        FT=cdiv(F, P),
    )
    return in_maps, meta


# --------------------------------------------------------------------------
# device program
# --------------------------------------------------------------------------

def build(meta, debug=False):
    N, F, D, K = meta["N"], meta["F"], meta["D"], meta["K"]
    NSH, NB, DT = meta["NSH"], meta["NB"], meta["DT"]
    rows_q = meta["rows_q"]
    PB = meta["PB"]
    CQB = meta["CQB"]
    NCHT = meta["NCHT"]
    strm_base = meta["strm_base"]
    Lq = meta["Lq"]
    ohbase = meta["ohbase"]
    FT = meta["FT"]
    W2 = 2 * D                  # combined table row elems (bf16)
    VLEN = 2 * D

    nc = bacc.Bacc("TRN2", target_bir_lowering=False, debug=debug,
                   num_devices=NC, num_swdge_queues=4,
                   dynamic_dma_scratch_size=65536)

    xt = nc.dram_tensor("xt", [F, NSH], fp8, kind="ExternalInput")
    xta = nc.dram_tensor("xta", [F, NSH], fp8, kind="ExternalInput")
    w1t = nc.dram_tensor("w1t", [F, D], bf16, kind="ExternalInput")
    wtt = nc.dram_tensor("wtt", [D, K], bf16, kind="ExternalInput")
    oh = nc.dram_tensor("oh", [P, NCHT, P], fp8, kind="ExternalInput")
    idx_d = [nc.dram_tensor(f"idx{q}", [P, Lq[q] // 16], i16,
                            kind="ExternalInput")
             for q in range(NQ)]
    gcnt_d = nc.dram_tensor("gcnt", [1, NQ * NB], i32, kind="ExternalInput")

    stats_v_d = nc.dram_tensor("stats_v", [1, VLEN], f32,
                               kind="ExternalOutput")
    stats_e_d = nc.dram_tensor("stats_e", [P, DT], f32, kind="ExternalOutput")
    s_out_d = nc.dram_tensor("s_out", [NB * P, K], f32, kind="ExternalOutput")

    max_grp = max((CQB[q][b] for b in range(NB) for q in range(NQ)),
                  default=1)
    gorder = [(q, b) for q in range(NQ) for b in range(NB) if CQB[q][b] > 0]

    # one-hot load batches in consumption (= emission) order
    batches = []          # (cstart, cend); group -> (batch idx, col offset)
    group_batch = {}
    for i in range(0, len(gorder), OH_BATCH):
        gs = gorder[i:i + OH_BATCH]
        cstart = ohbase[gs[0][0] * NB + gs[0][1]]
        cend = ohbase[gs[-1][0] * NB + gs[-1][1]] + CQB[gs[-1][0]][gs[-1][1]]
        bi = len(batches)
        batches.append((cstart, cend))
        for (q, b) in gs:
            group_batch[(q, b)] = (bi, ohbase[q * NB + b] - cstart)
    max_batch_nch = max((ce - cs for cs, ce in batches), default=1)

    with tile.TileContext(nc) as tc:
        with (
            tc.tile_pool(name="big", bufs=8) as bigp,
            tc.tile_pool(name="gtp", bufs=RING) as gtp,
            tc.tile_pool(name="ohp", bufs=2) as ohp,
            tc.tile_pool(name="persist", bufs=1) as persist,
            tc.tile_pool(name="stage", bufs=3) as stagep,
            tc.tile_pool(name="tmp", bufs=3) as tmpp,
            tc.tile_pool(name="small", bufs=4) as smallp,
            tc.tile_pool(name="svp", bufs=1) as svp,
            tc.tile_pool(name="pa", bufs=4, space="PSUM") as pa,
            tc.tile_pool(name="pb", bufs=3, space="PSUM") as pb,
            tc.tile_pool(name="red", bufs=1, space="PSUM") as redp,
            tc.tile_pool(name="dram", bufs=1, space="DRAM") as dramp,
        ):
            # ---- constants / resident tensors
            ident = persist.tile([P, P], f32)
            make_identity(nc, ident[:])
            ident_b = persist.tile([P, P], bf16, tag="identb")
            nc.vector.tensor_copy(ident_b[:], ident[:])
            w1t_t = persist.tile([P, FT, D], bf16)
            for t in range(FT):
                fr = min(P, F - t * P)
                nc.sync.dma_start(w1t_t[:fr, t, :], w1t[t * P:t * P + fr, :])
            wtt_t = persist.tile([P, DT, K], bf16)
            for t in range(DT):
                nc.sync.dma_start(wtt_t[:, t, :], wtt[t * P:(t + 1) * P, :])
            idx_t = []
            for q in range(NQ):
                it = persist.tile([P, Lq[q] // 16], i16, tag=f"idx{q}")
                nc.sync.dma_start(it[:], idx_d[q][:])
                idx_t.append(it)
            gcnt_t = persist.tile([1, NQ * NB], i32, tag="gcnt")
            nc.sync.dma_start(gcnt_t[:], gcnt_d[:])

            ln_la = persist.tile([P, 1], f32, tag="lnla")
            nc.vector.memset(ln_la[:], LN_SELU_LA)
            la_c = persist.tile([P, 1], f32, tag="lac")
            nc.vector.memset(la_c[:], SELU_LA)
            ones_b = persist.tile([P, 1], bf16, tag="onesb")
            nc.vector.memset(ones_b[:], 1.0)
            ones_f = persist.tile([P, 1], f32, tag="onesf")
            nc.vector.memset(ones_f[:], 1.0)
            # masked ones: 1.0 for partitions < last-block rows, else 0
            # (partition-offset memset is rejected by the verifier, so
            # build it as a row-slice reduction of the identity matrix)
            last_rows = NSH - (NB - 1) * P
            ones_mf = persist.tile([P, 1], f32, tag="onesmf")
            nc.vector.reduce_sum(ones_mf[:], ident[:, 0:last_rows],
                                 axis=mybir.AxisListType.X)
            ones_m = persist.tile([P, 1], bf16, tag="onesm")
            nc.vector.tensor_copy(ones_m[:], ones_mf[:])

            accs = persist.tile([P, VLEN], f32, tag="accs")
            nc.vector.memset(accs[:], 0.0)
            acc_blocks = persist.tile([P, NB, W2], bf16, tag="accb")

            red = redp.tile([P, DT], f32, tag="red")

            cc_in = [dramp.tile([rows_q[q], W2], fp8, name=f"cc_in{q}")
                     for q in range(NQ)]
            cc_out = [dramp.tile([NC * rows_q[q], W2], fp8,
                                 addr_space="Shared", name=f"cc_out{q}")
                      for q in range(NQ)]

            # ================= phase A: h1|h2 = [X|Xa] @ W1^T =============
            # one pass per AG chunk; each pass's rows AllGather as soon as
            # the pass's writes complete, so SpMM gathers on table 0 can
            # overlap the later AG chunks.
            for q in range(NQ):
                b_lo, b_hi = PB[q], PB[q + 1]
                c0 = b_lo * P
                c1 = min(b_hi * P, NSH)
                W = c1 - c0
                xt_tiles = []
                for which, src in enumerate((xt, xta)):
                    tl = []
                    for t in range(FT):
                        fr = min(P, F - t * P)
                        xx = bigp.tile([P, W], fp8, tag="big",
                                       name=f"xx{q}_{which}_{t}")
                        eng = nc.sync if (which * FT + t) % 2 == 0 else nc.scalar
                        eng.dma_start(xx[:fr, :], src[t * P:t * P + fr, c0:c1])
                        tl.append(xx)
                    xt_tiles.append(tl)
                for b in range(b_lo, b_hi):
                    rows = min(P, NSH - b * P)
                    off = b * P - c0
                    ptw = pb.tile([P, W2], f32, space="PSUM", tag="pb")
                    for which in range(2):
                        for t in range(FT):
                            fr = min(P, F - t * P)
                            nc.tensor.matmul(
                                ptw[:rows, which * D:(which + 1) * D],
                                lhsT=xt_tiles[which][t][:fr, off:off + rows],
                                rhs=w1t_t[:fr, t, :],
                                start=(t == 0), stop=(t == FT - 1),
                            )
                    st = stagep.tile([P, W2], fp8, tag="stage")
                    nc.vector.tensor_copy(st[:rows, :], ptw[:rows, :])
                    nc.scalar.dma_start(cc_in[q][b * P - c0:b * P - c0 + rows, :],
                                        st[:rows, :])
                nc.gpsimd.collective_compute(
                    "AllGather", mybir.AluOpType.bypass,
                    replica_groups=[list(range(NC))],
                    ins=[cc_in[q][:]], outs=[cc_out[q][:]],
                )

            # ================= gather stream (q-major emission) ===========
            gtile = {}
            prev_inst = None
            gq = 0
            cnt_reg = nc.gpsimd.alloc_register("gcnt_reg")
            for er, (q, b) in enumerate(gorder):
                sc = strm_base[q][b]
                n = CQB[q][b]
                gt = gtp.tile([P, max_grp, W2], fp8, tag="gt",
                              name=f"gt_{q}_{b}")
                if er < RING and n < max_grp:
                    # first ring pass leaves slices >= n uninitialized;
                    # zero them so later pad lanes never read NaN bits
                    nc.vector.memset(gt[:, n:max_grp, :], 0.0)
                ld = nc.gpsimd.reg_load(
                    cnt_reg, gcnt_t[0:1, q * NB + b:q * NB + b + 1])
                if prev_inst is not None:
                    tile.add_dep_helper(ld.ins, prev_inst, sync=False,
                                        reason="gather issue order")
                gi = nc.gpsimd.dma_gather(
                    gt[:, 0:n, :], cc_out[q][:],
                    idx_t[q][:, sc * 8:(sc + n) * 8],
                    num_idxs=n * P, num_idxs_reg=cnt_reg, elem_size=W2,
                    single_packet=False,
                    queue_num=gq % 4,
                )
                gq += 1
                tile.add_dep_helper(gi.ins, ld.ins, sync=False,
                                    reason="count reg load order")
                prev_inst = gi.ins
                gtile[(q, b)] = gt

            # ================= SpMM consumption + epilogues ===============
            oh_tiles = {}

            def get_oh(bi):
                if bi not in oh_tiles:
                    cs, ce = batches[bi]
                    oht = ohp.tile([P, max_batch_nch, P], fp8, tag="oh",
                                   name=f"oh_{bi}")
                    nc.scalar.dma_start(oht[:, 0:ce - cs, :],
                                        oh[:, cs:ce, :])
                    oh_tiles[bi] = oht
                return oh_tiles[bi]

            # issue the first few oh batch loads up front
            for bi in range(min(2, len(batches))):
                get_oh(bi)

            for q in range(NQ):
                for b in range(NB):
                    n = CQB[q][b]
                    rows = min(P, NSH - b * P)
                    accb = acc_blocks[:, b, :]
                    if n == 0:
                        if q == 0:
                            nc.vector.memset(accb, 0.0)
                        if q < NQ - 1:
                            continue
                    pt = None
                    if n > 0:
                        bi, coff = group_batch[(q, b)]
                        oht = get_oh(bi)
                        if bi + 1 < len(batches):
                            get_oh(bi + 1)
                        gt = gtile[(q, b)]
                        pt = pa.tile([P, W2], f32, space="PSUM", tag="pa")
                        for j in range(n):
                            nc.tensor.matmul(
                                pt[:], lhsT=oht[:, coff + j, :],
                                rhs=gt[:, j, :],
                                start=(j == 0), stop=(j == n - 1))
                    if q == 0:
                        nc.vector.tensor_copy(accb, pt[:])
                        continue
                    if q < NQ - 1:
                        nc.vector.tensor_tensor(accb, accb, pt[:],
                                                mybir.AluOpType.add)
                        continue

                    # ---- q == NQ-1: close the block ----
                    tot = tmpp.tile([P, W2], f32, tag="tot", bufs=2)
                    if pt is not None:
                        nc.vector.tensor_tensor(tot[:], pt[:], accb,
                                                mybir.AluOpType.add)
                    else:
                        nc.vector.tensor_copy(tot[:], accb)

                    # selu over the combined [128, 512] row:
                    # e2 = la*exp(x); e3 = relu(la - e2); r = relu(l*x)
                    # selu = r - e3   (bf16 out: the gathered table is fp8,
                    # so bf16 rounding here is noise)
                    e2 = tmpp.tile([P, W2], f32, tag="e2", bufs=2)
                    nc.scalar.activation(e2[:], tot[:],
                                         mybir.ActivationFunctionType.Exp,
                                         bias=ln_la[:])
                    e3 = tmpp.tile([P, W2], f32, tag="e3", bufs=2)
                    nc.scalar.activation(e3[:], e2[:],
                                         mybir.ActivationFunctionType.Relu,
                                         bias=la_c[:], scale=-1.0)
                    slu = tmpp.tile([P, W2], bf16, tag="slu")
                    nc.scalar.activation(slu[:], tot[:],
                                         mybir.ActivationFunctionType.Relu,
                                         scale=SELU_L)
                    nc.vector.tensor_tensor(slu[:], slu[:], e3[:],
                                            mybir.AluOpType.subtract)
                    g1 = slu[:, 0:D]
                    aug = slu[:, D:W2]

                    # con-loss partials
                    nc.vector.tensor_tensor(accs[:, 0:D], accs[:, 0:D], aug,
                                            mybir.AluOpType.add)
                    pr = tmpp.tile([P, D], bf16, tag="pr")
                    nc.vector.tensor_tensor(pr[:], aug, g1,
                                            mybir.AluOpType.mult)
                    nc.vector.tensor_tensor(accs[:, D:W2], accs[:, D:W2],
                                            pr[:], mybir.AluOpType.add)

                    # log-softmax stats: exp column-sums into persistent
                    # PSUM (no max subtraction needed; |g1| <~ 25)
                    eg = tmpp.tile([P, D], bf16, tag="eg", bufs=2)
                    nc.scalar.activation(eg[:], g1,
                                         mybir.ActivationFunctionType.Exp)
                    rvec = ones_m if rows < P else ones_b
                    for t in range(DT):
                        nc.tensor.matmul(red[:, t:t + 1],
                                         lhsT=eg[:, t * P:(t + 1) * P],
                                         rhs=rvec[:],
                                         start=(b == 0), stop=(b == NB - 1))

                    # S = softmax_K(g1 @ Wt^T)
                    g1T = tmpp.tile([P, D], bf16, tag="g1T")
                    for t in range(DT):
                        ptr = pb.tile([P, P], bf16, space="PSUM", tag="pb")
                        nc.tensor.transpose(ptr[:], g1[:, t * P:(t + 1) * P],
                                            ident_b[:])
                        nc.vector.tensor_copy(g1T[:, t * P:(t + 1) * P],
                                              ptr[:])
                    pl = pb.tile([P, K], f32, space="PSUM", tag="pb")
                    for t in range(DT):
                        nc.tensor.matmul(pl[:],
                                         lhsT=g1T[:, t * P:(t + 1) * P],
                                         rhs=wtt_t[:, t, :],
                                         start=(t == 0), stop=(t == DT - 1))
                    nmx = smallp.tile([P, 1], f32, tag="nmx")
                    nc.vector.reduce_max(nmx[:], pl[:],
                                         axis=mybir.AxisListType.X,
                                         negate=True)
                    ex = smallp.tile([P, K], f32, tag="ex")
                    sm = smallp.tile([P, 1], f32, tag="sm")
                    nc.scalar.activation(ex[:], pl[:],
                                         mybir.ActivationFunctionType.Exp,
                                         bias=nmx[:], accum_out=sm[:])
                    rc = smallp.tile([P, 1], f32, tag="rc")
                    nc.vector.reciprocal(rc[:], sm[:])
                    sb = stagep.tile([P, K], f32, tag="sstage")
                    nc.scalar.mul(sb[:], ex[:], rc[:])
                    nc.sync.dma_start(s_out_d[b * P:(b + 1) * P, :], sb[:])

            # ---- final tiny reductions out
            pv = pa.tile([P, VLEN], f32, space="PSUM", tag="pa")
            nc.tensor.matmul(pv[0:1, :], lhsT=ones_f[:], rhs=accs[:],
                             start=True, stop=True)
            sv = svp.tile([1, VLEN], f32, tag="sv")
            nc.vector.tensor_copy(sv[:], pv[0:1, :])
            nc.sync.dma_start(stats_v_d[:], sv[:])
            se = svp.tile([P, DT], f32, tag="se")
            nc.vector.tensor_copy(se[:], red[:])
            nc.sync.dma_start(stats_e_d[:], se[:])

    nc.compile()
    return nc


# --------------------------------------------------------------------------
# host-side combine of per-core partials
# --------------------------------------------------------------------------

def combine(results, cfg, graph_row, graph_col, graph_vals):
    N, D, K = cfg["N"], cfg["D"], cfg["K"]
    NSH = N // NC
    DT = D // P
    E = float(graph_row.shape[0])

    colsum_aug = np.zeros(D)
    dot = 0.0
    expsum = np.zeros(D)
    S_full = np.zeros((N, K))
    for c in range(NC):
        sv = np.asarray(results[c]["stats_v"], dtype=np.float64).reshape(-1)
        colsum_aug += sv[0:D]
        dot += sv[D:2 * D].sum()
        se = np.asarray(results[c]["stats_e"], dtype=np.float64)  # [P, DT]
        expsum += se.T.reshape(-1)          # d = t*128 + r
        S_full[c * NSH:(c + 1) * NSH] = \
            np.asarray(results[c]["s_out"], dtype=np.float64)[:NSH]
    logZ = np.log(expsum)

    row = np.asarray(graph_row).astype(np.int64)
    col = np.asarray(graph_col).astype(np.int64)
    av = np.asarray(graph_vals).astype(np.float64)
    deg = np.bincount(col, weights=av, minlength=N).astype(np.float64)

    trace_gp = np.einsum('e,ek,ek->', av, S_full[row], S_full[col])
    nl = S_full.T @ deg
    clsz = S_full.sum(axis=0)

    spectral = -(trace_gp - (nl ** 2).sum() / (2.0 * E)) / (2.0 * E)
    cluster = (np.linalg.norm(clsz) / N * math.sqrt(K) - 1.0) * CLUSTER_REG
    con = -(dot - (logZ * colsum_aug).sum()) / D
    return spectral + cluster + CON_REG * con


# --------------------------------------------------------------------------
# entry point
# --------------------------------------------------------------------------

_BUILD_CACHE = {}


def kernel(features, aug_features, graph_row, graph_col, graph_vals, gn_vals,
           lbl, dense_graph, W1, b1, Wt, bt, _cfg=None, _trace=False):
    cfg = _cfg or FULL
    in_maps, meta = prep(features, aug_features, graph_row, graph_col,
                         gn_vals, W1, Wt, cfg)
    key = tuple(sorted((k, str(v)) for k, v in meta.items()))
    if key not in _BUILD_CACHE:
        _BUILD_CACHE[key] = build(meta)
    nc = _BUILD_CACHE[key]
    res = bass_utils.run_bass_kernel_spmd(nc, in_maps, core_ids=list(range(NC)),
                                          trace=_trace)
    loss = combine(res.results, cfg, graph_row, graph_col, graph_vals)
    out = np.array(loss, dtype=np.float32)
    if _trace:
        return out, res
    return out


# revision 35
# speedup vs baseline: 1.4253x; 1.0090x over previous
"""Trainium2 8-core kernel for nn_CAT_81269371175150 (GNN message passing).

Math (see reference):
  gcn(x)   = selu(A_gn @ (x @ W1^T))            for features and aug_features
  S        = softmax_K(gcn1 @ Wt^T)
  loss     = spectral(S, A) + cluster(S) + 0.5 * con(gcn1, gcn2)

Strategy (v4: two-chunk pipelined AllGather over the fp8 v2 pipeline):
  * Nodes sharded row-wise across 8 cores.  Phase A computes
    h1|h2 = [X|Xa] @ W1^T in two block passes (17/32 blocks); each
    pass's rows are AllGathered as soon as they are ready, so the SpMM
    gather stream starts right after AG_0 (~115us earlier than a
    monolithic AG).  The second chunk is deliberately LARGE: the phase
    that closes blocks is gather-rich, so the per-block epilogues hide
    under gather time instead of piling up in a short tail.
  * Edges are bucketed by (source table q, dest block b) and gathered
    q-major.  Per (q,b) group: dma_gather of fp8 512B combined [h1|h2]
    rows (4 SWDGE queues, negative-index padding skip) + one fp8
    one-hot matmul per 128-edge chunk (gn folded in, host-built).
    Nothing but gather dispatch runs on gpsimd: its sequencer executes
    in order, so any epilogue op there would stall later desc-gens.
  * PSUM cannot hold 49 open accumulators, so q=0 partial sums spill to
    a bf16 SBUF accumulator per dest block (the gathered table is fp8,
    so bf16 partial accumulation error is negligible); q=1 closes each
    block: selu (single [128,512] chain), con-loss partials, S softmax.
  * log-softmax stats have no max subtraction (selu outputs are small
    enough that fp32/bf16 exp cannot overflow): exp column-sums
    accumulate into one persistent PSUM bank via tiny per-block matmuls,
    which kills the old gcn1T persistence + segmented-stats tail.
  * Host finishes the tiny reductions: trace(S^T A S), nl, cluster sizes,
    logZ across cores, final scalar.
"""

import math
import numpy as np
import ml_dtypes

import concourse.bacc as bacc
import concourse.mybir as mybir
import concourse.tile as tile
from concourse import bass_utils
from concourse.masks import make_identity

P = 128
NC = 8
NQ = 3                    # AllGather chunks / source tables
PASS_BLOCKS = (10, 17, 22)  # phase-A pass sizes in blocks (sum = NB = 49);
                          # every table is capped by int16 gather indices:
                          # 8 cores * 2794 rows = 22352 < 32768
RING = 8                  # gather tile ring depth
OH_BATCH = 3              # one-hot load batching (groups per dma)

# full-size problem constants
FULL = dict(N=50000, F=500, D=256, K=16)

SELU_L = 1.0507009873554805
SELU_A = 1.6732632423543772
SELU_LA = SELU_L * SELU_A
LN_SELU_LA = math.log(SELU_LA)

CLUSTER_REG = 1.0
CON_REG = 0.5

bf16 = mybir.dt.bfloat16
fp8 = mybir.dt.float8e4
f32 = mybir.dt.float32
i16 = mybir.dt.int16
i32 = mybir.dt.int32


def cdiv(a, b):
    return -(-a // b)


# --------------------------------------------------------------------------
# host-side preprocessing
# --------------------------------------------------------------------------

def prep(features, aug_features, graph_row, graph_col, gn_vals, W1, Wt, cfg):
    N, F, D, K = cfg["N"], cfg["F"], cfg["D"], cfg["K"]
    NSH = N // NC
    NB = cdiv(NSH, P)
    assert sum(PASS_BLOCKS) == NB

    # pass row offsets within a shard, and per-pass row counts
    PB = np.concatenate([[0], np.cumsum(PASS_BLOCKS)])        # block bounds
    RO = np.minimum(PB * P, NSH)                              # row bounds
    rows_q = [int(RO[q + 1] - RO[q]) for q in range(NQ)]

    row = np.asarray(graph_row).astype(np.int64)
    col = np.asarray(graph_col).astype(np.int64)
    gn = np.asarray(gn_vals).astype(np.float64)

    core = row // NSH
    per_core = []
    cnts = np.zeros((NC, NQ, NB), dtype=np.int64)
    for c in range(NC):
        m = core == c
        r = row[m] - c * NSH
        cl = col[m]
        g = gn[m]
        b = r // P
        src_core = cl // NSH
        off = cl % NSH
        q = np.digitize(off, RO[1:NQ])        # 0..NQ-1 by source pass
        loc = src_core * np.array(rows_q)[q] + (off - RO[q])
        order = np.lexsort((cl, b, q))
        r, cl, g, b, q, loc = (r[order], cl[order], g[order], b[order],
                               q[order], loc[order])
        key = q * NB + b
        cnt = np.bincount(key, minlength=NQ * NB).reshape(NQ, NB)
        cnts[c] = cnt
        per_core.append((r, g, b, q, loc, key))

    CQB = np.ceil(cnts.max(axis=0) / P).astype(np.int64)      # [NQ, NB]
    strm_base = np.zeros((NQ, NB), dtype=np.int64)
    for q in range(NQ):
        strm_base[q] = np.concatenate([[0], np.cumsum(CQB[q])[:-1]])
    Lq = [int(CQB[q].sum()) * P for q in range(NQ)]
    nch_flat = CQB.reshape(-1)                                # q-major
    ohbase = np.concatenate([[0], np.cumsum(nch_flat)[:-1]])
    NCHT = int(nch_flat.sum())
    emitted = [(q, b) for q in range(NQ) for b in range(NB)
               if CQB[q][b] > 0]
    emit_rank = {g: i for i, g in enumerate(emitted)}

    X = np.asarray(features)[0]
    Xa = np.asarray(aug_features)[0]
    XT = np.ascontiguousarray(X.T).astype(ml_dtypes.float8_e4m3)   # [F, N]
    XTa = np.ascontiguousarray(Xa.T).astype(ml_dtypes.float8_e4m3)
    W1T = np.ascontiguousarray(np.asarray(W1).T).astype(ml_dtypes.bfloat16)
    WtT = np.ascontiguousarray(np.asarray(Wt).T).astype(ml_dtypes.bfloat16)

    def wrap_idx(a):
        # [L] -> [128, L/16]: element i at [i%16, i//16], replicated x8
        L = a.shape[0]
        w = a.reshape(L // 16, 16).T
        return np.ascontiguousarray(np.tile(w, (8, 1)))

    in_maps = []
    for c in range(NC):
        r, g, b, q, loc, key = per_core[c]
        cnt = cnts[c]
        run_start = np.zeros(NQ * NB, dtype=np.int64)
        flat = cnt.reshape(-1)
        run_start[1:] = np.cumsum(flat)[:-1]
        rank = np.arange(len(r)) - run_start[key]
        lane = rank % P
        j = rank // P

        idx_streams = []
        gcnt = np.zeros(NQ * NB, dtype=np.int32)
        for qq in range(NQ):
            arr = np.full(Lq[qq], -1, dtype=np.int16)
            m = q == qq
            offp = (strm_base[qq][b[m]] + j[m]) * P + lane[m]
            arr[offp] = loc[m].astype(np.int16)
            for bb in range(NB):
                n = CQB[qq][bb]
                if n == 0:
                    continue
                base = strm_base[qq][bb] * P
                cv = int(cnt[qq][bb])
                if emit_rank.get((qq, bb), 0) < RING:
                    eff = n * P          # first ring pass: gather everything
                else:
                    eff = min(max(cdiv(cv, 16) * 16, P), n * P)
                arr[base + cv:base + eff] = 0
                gcnt[qq * NB + bb] = eff
            idx_streams.append(wrap_idx(arr))

        oh = np.zeros((P, NCHT, P), dtype=ml_dtypes.float8_e4m3)
        ohcol = ohbase[key] + j
        dest = r - b * P
        oh[lane, ohcol, dest] = g.astype(ml_dtypes.float8_e4m3)

        im = {
            "xt": np.ascontiguousarray(XT[:, c * NSH:(c + 1) * NSH]),
            "xta": np.ascontiguousarray(XTa[:, c * NSH:(c + 1) * NSH]),
            "w1t": W1T,
            "wtt": WtT,
            "oh": oh,
            "gcnt": gcnt.reshape(1, -1),
        }
        for q in range(NQ):
            im[f"idx{q}"] = idx_streams[q]
        in_maps.append(im)

    meta = dict(
        N=N, F=F, D=D, K=K, NSH=NSH, NB=NB, DT=D // P,
        rows_q=tuple(rows_q), PB=tuple(int(x) for x in PB),
        CQB=tuple(map(tuple, CQB.tolist())), NCHT=NCHT,
        strm_base=tuple(map(tuple, strm_base.tolist())),
        Lq=tuple(Lq), ohbase=tuple(int(x) for x in ohbase),
        FT=cdiv(F, P),
    )
    return in_maps, meta


# --------------------------------------------------------------------------
# device program
# --------------------------------------------------------------------------

def build(meta, debug=False):
    N, F, D, K = meta["N"], meta["F"], meta["D"], meta["K"]
    NSH, NB, DT = meta["NSH"], meta["NB"], meta["DT"]
    rows_q = meta["rows_q"]
    PB = meta["PB"]
    CQB = meta["CQB"]
    NCHT = meta["NCHT"]
    strm_base = meta["strm_base"]
    Lq = meta["Lq"]
    ohbase = meta["ohbase"]
    FT = meta["FT"]
    W2 = 2 * D                  # combined table row elems (bf16)
    VLEN = 2 * D

    nc = bacc.Bacc("TRN2", target_bir_lowering=False, debug=debug,
                   num_devices=NC, num_swdge_queues=4,
                   dynamic_dma_scratch_size=32768)

    xt = nc.dram_tensor("xt", [F, NSH], fp8, kind="ExternalInput")
    xta = nc.dram_tensor("xta", [F, NSH], fp8, kind="ExternalInput")
    w1t = nc.dram_tensor("w1t", [F, D], bf16, kind="ExternalInput")
    wtt = nc.dram_tensor("wtt", [D, K], bf16, kind="ExternalInput")
    oh = nc.dram_tensor("oh", [P, NCHT, P], fp8, kind="ExternalInput")
    idx_d = [nc.dram_tensor(f"idx{q}", [P, Lq[q] // 16], i16,
                            kind="ExternalInput")
             for q in range(NQ)]
    gcnt_d = nc.dram_tensor("gcnt", [1, NQ * NB], i32, kind="ExternalInput")

    stats_v_d = nc.dram_tensor("stats_v", [1, VLEN], f32,
                               kind="ExternalOutput")
    stats_e_d = nc.dram_tensor("stats_e", [P, DT], f32, kind="ExternalOutput")
    s_out_d = nc.dram_tensor("s_out", [NB * P, K], f32, kind="ExternalOutput")

    max_grp = max((CQB[q][b] for b in range(NB) for q in range(NQ)),
                  default=1)
    gorder = [(q, b) for q in range(NQ) for b in range(NB) if CQB[q][b] > 0]

    # one-hot load batches in consumption (= emission) order
    batches = []          # (cstart, cend); group -> (batch idx, col offset)
    group_batch = {}
    for i in range(0, len(gorder), OH_BATCH):
        gs = gorder[i:i + OH_BATCH]
        cstart = ohbase[gs[0][0] * NB + gs[0][1]]
        cend = ohbase[gs[-1][0] * NB + gs[-1][1]] + CQB[gs[-1][0]][gs[-1][1]]
        bi = len(batches)
        batches.append((cstart, cend))
        for (q, b) in gs:
            group_batch[(q, b)] = (bi, ohbase[q * NB + b] - cstart)
    max_batch_nch = max((ce - cs for cs, ce in batches), default=1)

    with tile.TileContext(nc) as tc:
        with (
            tc.tile_pool(name="big", bufs=8) as bigp,
            tc.tile_pool(name="gtp", bufs=RING) as gtp,
            tc.tile_pool(name="ohp", bufs=3) as ohp,
            tc.tile_pool(name="persist", bufs=1) as persist,
            tc.tile_pool(name="stage", bufs=3) as stagep,
            tc.tile_pool(name="tmp", bufs=3) as tmpp,
            tc.tile_pool(name="small", bufs=4) as smallp,
            tc.tile_pool(name="svp", bufs=1) as svp,
            tc.tile_pool(name="pa", bufs=4, space="PSUM") as pa,
            tc.tile_pool(name="pb", bufs=3, space="PSUM") as pb,
            tc.tile_pool(name="red", bufs=1, space="PSUM") as redp,
            tc.tile_pool(name="dram", bufs=1, space="DRAM") as dramp,
        ):
            # ---- constants / resident tensors
            ident = persist.tile([P, P], f32)
            make_identity(nc, ident[:])
            ident_b = persist.tile([P, P], bf16, tag="identb")
            nc.vector.tensor_copy(ident_b[:], ident[:])
            w1t_t = persist.tile([P, FT, D], bf16)
            for t in range(FT):
                fr = min(P, F - t * P)
                nc.sync.dma_start(w1t_t[:fr, t, :], w1t[t * P:t * P + fr, :])
            wtt_t = persist.tile([P, DT, K], bf16)
            for t in range(DT):
                nc.sync.dma_start(wtt_t[:, t, :], wtt[t * P:(t + 1) * P, :])

            ln_la = persist.tile([P, 1], f32, tag="lnla")
            nc.vector.memset(ln_la[:], LN_SELU_LA)
            la_c = persist.tile([P, 1], f32, tag="lac")
            nc.vector.memset(la_c[:], SELU_LA)
            ones_b = persist.tile([P, 1], bf16, tag="onesb")
            nc.vector.memset(ones_b[:], 1.0)
            ones_f = persist.tile([P, 1], f32, tag="onesf")
            nc.vector.memset(ones_f[:], 1.0)
            # masked ones: 1.0 for partitions < last-block rows, else 0
            # (partition-offset memset is rejected by the verifier, so
            # build it as a row-slice reduction of the identity matrix)
            last_rows = NSH - (NB - 1) * P
            ones_mf = persist.tile([P, 1], f32, tag="onesmf")
            nc.vector.reduce_sum(ones_mf[:], ident[:, 0:last_rows],
                                 axis=mybir.AxisListType.X)
            ones_m = persist.tile([P, 1], bf16, tag="onesm")
            nc.vector.tensor_copy(ones_m[:], ones_mf[:])

            accs = persist.tile([P, VLEN], f32, tag="accs")
            nc.vector.memset(accs[:], 0.0)
            acc_blocks = persist.tile([P, NB, W2], bf16, tag="accb")

            red = redp.tile([P, DT], f32, tag="red")

            cc_in = [dramp.tile([rows_q[q], W2], fp8, name=f"cc_in{q}")
                     for q in range(NQ)]
            cc_out = [dramp.tile([NC * rows_q[q], W2], fp8,
                                 addr_space="Shared", name=f"cc_out{q}")
                      for q in range(NQ)]

            # ================= phase A: h1|h2 = [X|Xa] @ W1^T =============
            # one pass per AG chunk; each pass's rows AllGather as soon as
            # the pass's writes complete, so SpMM gathers on table 0 can
            # overlap the later AG chunks.  xx loads live on the scalar
            # HWDGE ring and st writes on the sync ring: sharing one ring
            # queues the st writes behind megabytes of feature loads and
            # delays the AG triggers by ~50us.
            idx_t = []
            for q in range(NQ):
                b_lo, b_hi = PB[q], PB[q + 1]
                c0 = b_lo * P
                c1 = min(b_hi * P, NSH)
                W = c1 - c0
                xt_tiles = []
                for which, src in enumerate((xt, xta)):
                    tl = []
                    for t in range(FT):
                        fr = min(P, F - t * P)
                        xx = bigp.tile([P, W], fp8, tag="big",
                                       name=f"xx{q}_{which}_{t}")
                        nc.scalar.dma_start(xx[:fr, :],
                                            src[t * P:t * P + fr, c0:c1])
                        tl.append(xx)
                    xt_tiles.append(tl)
                for b in range(b_lo, b_hi):
                    rows = min(P, NSH - b * P)
                    off = b * P - c0
                    ptw = pb.tile([P, W2], f32, space="PSUM", tag="pb")
                    for which in range(2):
                        for t in range(FT):
                            fr = min(P, F - t * P)
                            nc.tensor.matmul(
                                ptw[:rows, which * D:(which + 1) * D],
                                lhsT=xt_tiles[which][t][:fr, off:off + rows],
                                rhs=w1t_t[:fr, t, :],
                                start=(t == 0), stop=(t == FT - 1),
                            )
                    st = stagep.tile([P, W2], fp8, tag="stage")
                    nc.vector.tensor_copy(st[:rows, :], ptw[:rows, :])
                    nc.sync.dma_start(cc_in[q][b * P - c0:b * P - c0 + rows, :],
                                      st[:rows, :])
                nc.gpsimd.collective_compute(
                    "AllGather", mybir.AluOpType.bypass,
                    replica_groups=[list(range(NC))],
                    ins=[cc_in[q][:]], outs=[cc_out[q][:]],
                )
                # gather metadata for table q: emitted after pass q's
                # cc_in writes so it never delays the AG trigger, but
                # early enough to be resident before its gathers start
                if q == 0:
                    gcnt_t = persist.tile([1, NQ * NB], i32, tag="gcnt")
                    nc.sync.dma_start(gcnt_t[:], gcnt_d[:])
                it = persist.tile([P, Lq[q] // 16], i16, tag=f"idx{q}")
                nc.sync.dma_start(it[:], idx_d[q][:])
                idx_t.append(it)

            # ================= gather stream (q-major emission) ===========
            gtile = {}
            prev_inst = None
            gq = 0
            cnt_reg = nc.gpsimd.alloc_register("gcnt_reg")
            for er, (q, b) in enumerate(gorder):
                sc = strm_base[q][b]
                n = CQB[q][b]
                gt = gtp.tile([P, max_grp, W2], fp8, tag="gt",
                              name=f"gt_{q}_{b}")
                if er < RING and n < max_grp:
                    # first ring pass leaves slices >= n uninitialized;
                    # zero them so later pad lanes never read NaN bits
                    nc.vector.memset(gt[:, n:max_grp, :], 0.0)
                ld = nc.gpsimd.reg_load(
                    cnt_reg, gcnt_t[0:1, q * NB + b:q * NB + b + 1])
                if prev_inst is not None:
                    tile.add_dep_helper(ld.ins, prev_inst, sync=False,
                                        reason="gather issue order")
                gi = nc.gpsimd.dma_gather(
                    gt[:, 0:n, :], cc_out[q][:],
                    idx_t[q][:, sc * 8:(sc + n) * 8],
                    num_idxs=n * P, num_idxs_reg=cnt_reg, elem_size=W2,
                    single_packet=False,
                    queue_num=gq % 4,
                )
                gq += 1
                tile.add_dep_helper(gi.ins, ld.ins, sync=False,
                                    reason="count reg load order")
                prev_inst = gi.ins
                gtile[(q, b)] = gt

            # ================= SpMM consumption + epilogues ===============
            oh_tiles = {}

            def get_oh(bi):
                if bi not in oh_tiles:
                    cs, ce = batches[bi]
                    oht = ohp.tile([P, max_batch_nch, P], fp8, tag="oh",
                                   name=f"oh_{bi}")
                    nc.scalar.dma_start(oht[:, 0:ce - cs, :],
                                        oh[:, cs:ce, :])
                    oh_tiles[bi] = oht
                return oh_tiles[bi]

            # issue the first few oh batch loads up front
            for bi in range(min(2, len(batches))):
                get_oh(bi)

            for q in range(NQ):
                for b in range(NB):
                    n = CQB[q][b]
                    rows = min(P, NSH - b * P)
                    accb = acc_blocks[:, b, :]
                    if n == 0:
                        if q == 0:
                            nc.vector.memset(accb, 0.0)
                        if q < NQ - 1:
                            continue
                    pt = None
                    if n > 0:
                        bi, coff = group_batch[(q, b)]
                        oht = get_oh(bi)
                        if bi + 1 < len(batches):
                            get_oh(bi + 1)
                        gt = gtile[(q, b)]
                        pt = pa.tile([P, W2], f32, space="PSUM", tag="pa")
                        for j in range(n):
                            nc.tensor.matmul(
                                pt[:], lhsT=oht[:, coff + j, :],
                                rhs=gt[:, j, :],
                                start=(j == 0), stop=(j == n - 1))
                    if q == 0:
                        # scalar engine is idle in the q0 window; DVE is not
                        nc.scalar.activation(accb, pt[:],
                                             mybir.ActivationFunctionType.Copy)
                        continue
                    if q < NQ - 1:
                        nc.vector.tensor_tensor(accb, accb, pt[:],
                                                mybir.AluOpType.add)
                        continue

                    # ---- q == NQ-1: close the block ----
                    tot = tmpp.tile([P, W2], f32, tag="tot")
                    if pt is not None:
                        nc.vector.tensor_tensor(tot[:], pt[:], accb,
                                                mybir.AluOpType.add)
                    else:
                        nc.vector.tensor_copy(tot[:], accb)

                    # selu over the combined [128, 512] row:
                    # e2 = la*exp(x); e3 = relu(la - e2); r = relu(l*x)
                    # selu = r - e3   (bf16 out: the gathered table is fp8,
                    # so bf16 rounding here is noise)
                    e2 = tmpp.tile([P, W2], f32, tag="e2")
                    nc.scalar.activation(e2[:], tot[:],
                                         mybir.ActivationFunctionType.Exp,
                                         bias=ln_la[:])
                    e3 = tmpp.tile([P, W2], f32, tag="e3")
                    nc.scalar.activation(e3[:], e2[:],
                                         mybir.ActivationFunctionType.Relu,
                                         bias=la_c[:], scale=-1.0)
                    slu = tmpp.tile([P, W2], bf16, tag="slu")
                    nc.scalar.activation(slu[:], tot[:],
                                         mybir.ActivationFunctionType.Relu,
                                         scale=SELU_L)
                    nc.vector.tensor_tensor(slu[:], slu[:], e3[:],
                                            mybir.AluOpType.subtract)
                    g1 = slu[:, 0:D]
                    aug = slu[:, D:W2]

                    # con-loss partials
                    nc.vector.tensor_tensor(accs[:, 0:D], accs[:, 0:D], aug,
                                            mybir.AluOpType.add)
                    pr = tmpp.tile([P, D], bf16, tag="pr")
                    nc.vector.tensor_tensor(pr[:], aug, g1,
                                            mybir.AluOpType.mult)
                    nc.vector.tensor_tensor(accs[:, D:W2], accs[:, D:W2],
                                            pr[:], mybir.AluOpType.add)

                    # log-softmax stats: exp column-sums into persistent
                    # PSUM (no max subtraction needed; |g1| <~ 25)
                    eg = tmpp.tile([P, D], bf16, tag="eg")
                    nc.scalar.activation(eg[:], g1,
                                         mybir.ActivationFunctionType.Exp)
                    rvec = ones_m if rows < P else ones_b
                    for t in range(DT):
                        nc.tensor.matmul(red[:, t:t + 1],
                                         lhsT=eg[:, t * P:(t + 1) * P],
                                         rhs=rvec[:],
                                         start=(b == 0), stop=(b == NB - 1))

                    # S = softmax_K(g1 @ Wt^T)
                    g1T = tmpp.tile([P, D], bf16, tag="g1T")
                    for t in range(DT):
                        ptr = pb.tile([P, P], bf16, space="PSUM", tag="pb")
                        nc.tensor.transpose(ptr[:], g1[:, t * P:(t + 1) * P],
                                            ident_b[:])
                        nc.vector.tensor_copy(g1T[:, t * P:(t + 1) * P],
                                              ptr[:])
                    pl = pb.tile([P, K], f32, space="PSUM", tag="pb")
                    for t in range(DT):
                        nc.tensor.matmul(pl[:],
                                         lhsT=g1T[:, t * P:(t + 1) * P],
                                         rhs=wtt_t[:, t, :],
                                         start=(t == 0), stop=(t == DT - 1))
                    # no max subtraction: |logits| <~ 30, exp is f32-safe
                    ex = smallp.tile([P, K], f32, tag="ex")
                    sm = smallp.tile([P, 1], f32, tag="sm")
                    nc.scalar.activation(ex[:], pl[:],
                                         mybir.ActivationFunctionType.Exp,
                                         accum_out=sm[:])
                    rc = smallp.tile([P, 1], f32, tag="rc")
                    nc.vector.reciprocal(rc[:], sm[:])
                    sb = stagep.tile([P, K], f32, tag="sstage")
                    nc.scalar.mul(sb[:], ex[:], rc[:])
                    nc.sync.dma_start(s_out_d[b * P:(b + 1) * P, :], sb[:])

            # ---- final tiny reductions out
            pv = pa.tile([P, VLEN], f32, space="PSUM", tag="pa")
            nc.tensor.matmul(pv[0:1, :], lhsT=ones_f[:], rhs=accs[:],
                             start=True, stop=True)
            sv = svp.tile([1, VLEN], f32, tag="sv")
            nc.vector.tensor_copy(sv[:], pv[0:1, :])
            nc.sync.dma_start(stats_v_d[:], sv[:])
            se = svp.tile([P, DT], f32, tag="se")
            nc.vector.tensor_copy(se[:], red[:])
            nc.sync.dma_start(stats_e_d[:], se[:])

    nc.compile()
    return nc


# --------------------------------------------------------------------------
# host-side combine of per-core partials
# --------------------------------------------------------------------------

def combine(results, cfg, graph_row, graph_col, graph_vals):
    N, D, K = cfg["N"], cfg["D"], cfg["K"]
    NSH = N // NC
    DT = D // P
    E = float(graph_row.shape[0])

    colsum_aug = np.zeros(D)
    dot = 0.0
    expsum = np.zeros(D)
    S_full = np.zeros((N, K))
    for c in range(NC):
        sv = np.asarray(results[c]["stats_v"], dtype=np.float64).reshape(-1)
        colsum_aug += sv[0:D]
        dot += sv[D:2 * D].sum()
        se = np.asarray(results[c]["stats_e"], dtype=np.float64)  # [P, DT]
        expsum += se.T.reshape(-1)          # d = t*128 + r
        S_full[c * NSH:(c + 1) * NSH] = \
            np.asarray(results[c]["s_out"], dtype=np.float64)[:NSH]
    logZ = np.log(expsum)

    row = np.asarray(graph_row).astype(np.int64)
    col = np.asarray(graph_col).astype(np.int64)
    av = np.asarray(graph_vals).astype(np.float64)
    deg = np.bincount(col, weights=av, minlength=N).astype(np.float64)

    trace_gp = np.einsum('e,ek,ek->', av, S_full[row], S_full[col])
    nl = S_full.T @ deg
    clsz = S_full.sum(axis=0)

    spectral = -(trace_gp - (nl ** 2).sum() / (2.0 * E)) / (2.0 * E)
    cluster = (np.linalg.norm(clsz) / N * math.sqrt(K) - 1.0) * CLUSTER_REG
    con = -(dot - (logZ * colsum_aug).sum()) / D
    return spectral + cluster + CON_REG * con


# --------------------------------------------------------------------------
# entry point
# --------------------------------------------------------------------------

_BUILD_CACHE = {}


def kernel(features, aug_features, graph_row, graph_col, graph_vals, gn_vals,
           lbl, dense_graph, W1, b1, Wt, bt, _cfg=None, _trace=False):
    cfg = _cfg or FULL
    in_maps, meta = prep(features, aug_features, graph_row, graph_col,
                         gn_vals, W1, Wt, cfg)
    key = tuple(sorted((k, str(v)) for k, v in meta.items()))
    if key not in _BUILD_CACHE:
        _BUILD_CACHE[key] = build(meta)
    nc = _BUILD_CACHE[key]
    res = bass_utils.run_bass_kernel_spmd(nc, in_maps, core_ids=list(range(NC)),
                                          trace=_trace)
    loss = combine(res.results, cfg, graph_row, graph_col, graph_vals)
    out = np.array(loss, dtype=np.float32)
    if _trace:
        return out, res
    return out


# revision 38
# speedup vs baseline: 1.4552x; 1.0209x over previous
"""Trainium2 8-core kernel for nn_CAT_81269371175150 (GNN message passing).

Math (see reference):
  gcn(x)   = selu(A_gn @ (x @ W1^T))            for features and aug_features
  S        = softmax_K(gcn1 @ Wt^T)
  loss     = spectral(S, A) + cluster(S) + 0.5 * con(gcn1, gcn2)

Strategy (v4: two-chunk pipelined AllGather over the fp8 v2 pipeline):
  * Nodes sharded row-wise across 8 cores.  Phase A computes
    h1|h2 = [X|Xa] @ W1^T in two block passes (17/32 blocks); each
    pass's rows are AllGathered as soon as they are ready, so the SpMM
    gather stream starts right after AG_0 (~115us earlier than a
    monolithic AG).  The second chunk is deliberately LARGE: the phase
    that closes blocks is gather-rich, so the per-block epilogues hide
    under gather time instead of piling up in a short tail.
  * Edges are bucketed by (source table q, dest block b) and gathered
    q-major.  Per (q,b) group: dma_gather of fp8 512B combined [h1|h2]
    rows (4 SWDGE queues, negative-index padding skip) + one fp8
    one-hot matmul per 128-edge chunk (gn folded in, host-built).
    Nothing but gather dispatch runs on gpsimd: its sequencer executes
    in order, so any epilogue op there would stall later desc-gens.
  * PSUM cannot hold 49 open accumulators, so q=0 partial sums spill to
    a bf16 SBUF accumulator per dest block (the gathered table is fp8,
    so bf16 partial accumulation error is negligible); q=1 closes each
    block: selu (single [128,512] chain), con-loss partials, S softmax.
  * log-softmax stats have no max subtraction (selu outputs are small
    enough that fp32/bf16 exp cannot overflow): exp column-sums
    accumulate into one persistent PSUM bank via tiny per-block matmuls,
    which kills the old gcn1T persistence + segmented-stats tail.
  * Host finishes the tiny reductions: trace(S^T A S), nl, cluster sizes,
    logZ across cores, final scalar.
"""

import math
import numpy as np
import ml_dtypes

import concourse.bacc as bacc
import concourse.mybir as mybir
import concourse.tile as tile
from concourse import bass_utils
from concourse.masks import make_identity

P = 128
NC = 8
NQ = 2                    # AllGather chunks / source tables
PASS_BLOCKS = (17, 32)    # phase-A pass sizes in blocks (sum = NB = 49);
                          # pass 1 is capped by int16 gather indices:
                          # 8 cores * 4074 rows = 32592 < 32768
RING = 7                  # gather tile ring depth
OH_BATCH = 3              # one-hot load batching (groups per dma)

# full-size problem constants
FULL = dict(N=50000, F=500, D=256, K=16)

SELU_L = 1.0507009873554805
SELU_A = 1.6732632423543772
SELU_LA = SELU_L * SELU_A
LN_SELU_LA = math.log(SELU_LA)

CLUSTER_REG = 1.0
CON_REG = 0.5

bf16 = mybir.dt.bfloat16
fp8 = mybir.dt.float8e4
f32 = mybir.dt.float32
i16 = mybir.dt.int16
i32 = mybir.dt.int32


def cdiv(a, b):
    return -(-a // b)


# --------------------------------------------------------------------------
# host-side preprocessing
# --------------------------------------------------------------------------

def prep(features, aug_features, graph_row, graph_col, gn_vals, W1, Wt, cfg):
    N, F, D, K = cfg["N"], cfg["F"], cfg["D"], cfg["K"]
    NSH = N // NC
    NB = cdiv(NSH, P)
    assert sum(PASS_BLOCKS) == NB

    # pass row offsets within a shard, and per-pass row counts
    PB = np.concatenate([[0], np.cumsum(PASS_BLOCKS)])        # block bounds
    RO = np.minimum(PB * P, NSH)                              # row bounds
    rows_q = [int(RO[q + 1] - RO[q]) for q in range(NQ)]

    row = np.asarray(graph_row).astype(np.int64)
    col = np.asarray(graph_col).astype(np.int64)
    gn = np.asarray(gn_vals).astype(np.float64)

    core = row // NSH
    per_core = []
    cnts = np.zeros((NC, NQ, NB), dtype=np.int64)
    for c in range(NC):
        m = core == c
        r = row[m] - c * NSH
        cl = col[m]
        g = gn[m]
        b = r // P
        src_core = cl // NSH
        off = cl % NSH
        q = np.digitize(off, RO[1:NQ])        # 0..NQ-1 by source pass
        loc = src_core * np.array(rows_q)[q] + (off - RO[q])
        order = np.lexsort((cl, b, q))
        r, cl, g, b, q, loc = (r[order], cl[order], g[order], b[order],
                               q[order], loc[order])
        key = q * NB + b
        cnt = np.bincount(key, minlength=NQ * NB).reshape(NQ, NB)
        cnts[c] = cnt
        per_core.append((r, g, b, q, loc, key))

    CQB = np.ceil(cnts.max(axis=0) / P).astype(np.int64)      # [NQ, NB]
    strm_base = np.zeros((NQ, NB), dtype=np.int64)
    for q in range(NQ):
        strm_base[q] = np.concatenate([[0], np.cumsum(CQB[q])[:-1]])
    Lq = [int(CQB[q].sum()) * P for q in range(NQ)]
    nch_flat = CQB.reshape(-1)                                # q-major
    ohbase = np.concatenate([[0], np.cumsum(nch_flat)[:-1]])
    NCHT = int(nch_flat.sum())
    emitted = [(q, b) for q in range(NQ) for b in range(NB)
               if CQB[q][b] > 0]
    emit_rank = {g: i for i, g in enumerate(emitted)}

    X = np.asarray(features)[0]
    Xa = np.asarray(aug_features)[0]
    XT = np.ascontiguousarray(X.T).astype(ml_dtypes.float8_e4m3)   # [F, N]
    XTa = np.ascontiguousarray(Xa.T).astype(ml_dtypes.float8_e4m3)
    W1T = np.ascontiguousarray(np.asarray(W1).T).astype(ml_dtypes.bfloat16)
    WtT = np.ascontiguousarray(np.asarray(Wt).T).astype(ml_dtypes.bfloat16)

    def wrap_idx(a):
        # [L] -> [128, L/16]: element i at [i%16, i//16], replicated x8
        L = a.shape[0]
        w = a.reshape(L // 16, 16).T
        return np.ascontiguousarray(np.tile(w, (8, 1)))

    in_maps = []
    for c in range(NC):
        r, g, b, q, loc, key = per_core[c]
        cnt = cnts[c]
        run_start = np.zeros(NQ * NB, dtype=np.int64)
        flat = cnt.reshape(-1)
        run_start[1:] = np.cumsum(flat)[:-1]
        rank = np.arange(len(r)) - run_start[key]
        lane = rank % P
        j = rank // P

        idx_streams = []
        gcnt = np.zeros(NQ * NB, dtype=np.int32)
        for qq in range(NQ):
            arr = np.full(Lq[qq], -1, dtype=np.int16)
            m = q == qq
            offp = (strm_base[qq][b[m]] + j[m]) * P + lane[m]
            arr[offp] = loc[m].astype(np.int16)
            for bb in range(NB):
                n = CQB[qq][bb]
                if n == 0:
                    continue
                base = strm_base[qq][bb] * P
                cv = int(cnt[qq][bb])
                if emit_rank.get((qq, bb), 0) < RING:
                    eff = n * P          # first ring pass: gather everything
                else:
                    eff = min(max(cdiv(cv, 16) * 16, P), n * P)
                arr[base + cv:base + eff] = 0
                gcnt[qq * NB + bb] = eff
            idx_streams.append(wrap_idx(arr))

        oh = np.zeros((P, NCHT, P), dtype=ml_dtypes.float8_e4m3)
        ohcol = ohbase[key] + j
        dest = r - b * P
        oh[lane, ohcol, dest] = g.astype(ml_dtypes.float8_e4m3)

        im = {
            "xt": np.ascontiguousarray(XT[:, c * NSH:(c + 1) * NSH]),
            "xta": np.ascontiguousarray(XTa[:, c * NSH:(c + 1) * NSH]),
            "w1t": W1T,
            "wtt": WtT,
            "oh": oh,
            "gcnt": gcnt.reshape(1, -1),
        }
        for q in range(NQ):
            im[f"idx{q}"] = idx_streams[q]
        in_maps.append(im)

    meta = dict(
        N=N, F=F, D=D, K=K, NSH=NSH, NB=NB, DT=D // P,
        rows_q=tuple(rows_q), PB=tuple(int(x) for x in PB),
        CQB=tuple(map(tuple, CQB.tolist())), NCHT=NCHT,
        strm_base=tuple(map(tuple, strm_base.tolist())),
        Lq=tuple(Lq), ohbase=tuple(int(x) for x in ohbase),
        FT=cdiv(F, P),
    )
    return in_maps, meta


# --------------------------------------------------------------------------
# device program
# --------------------------------------------------------------------------

def build(meta, debug=False):
    N, F, D, K = meta["N"], meta["F"], meta["D"], meta["K"]
    NSH, NB, DT = meta["NSH"], meta["NB"], meta["DT"]
    rows_q = meta["rows_q"]
    PB = meta["PB"]
    CQB = meta["CQB"]
    NCHT = meta["NCHT"]
    strm_base = meta["strm_base"]
    Lq = meta["Lq"]
    ohbase = meta["ohbase"]
    FT = meta["FT"]
    W2 = 2 * D                  # combined table row elems (bf16)
    VLEN = 2 * D

    nc = bacc.Bacc("TRN2", target_bir_lowering=False, debug=debug,
                   num_devices=NC, num_swdge_queues=4,
                   dynamic_dma_scratch_size=32768)

    xt = nc.dram_tensor("xt", [F, NSH], fp8, kind="ExternalInput")
    xta = nc.dram_tensor("xta", [F, NSH], fp8, kind="ExternalInput")
    w1t = nc.dram_tensor("w1t", [F, D], bf16, kind="ExternalInput")
    wtt = nc.dram_tensor("wtt", [D, K], bf16, kind="ExternalInput")
    oh = nc.dram_tensor("oh", [P, NCHT, P], fp8, kind="ExternalInput")
    idx_d = [nc.dram_tensor(f"idx{q}", [P, Lq[q] // 16], i16,
                            kind="ExternalInput")
             for q in range(NQ)]
    gcnt_d = nc.dram_tensor("gcnt", [1, NQ * NB], i32, kind="ExternalInput")

    stats_v_d = nc.dram_tensor("stats_v", [1, VLEN], f32,
                               kind="ExternalOutput")
    stats_e_d = nc.dram_tensor("stats_e", [P, DT], f32, kind="ExternalOutput")
    s_out_d = nc.dram_tensor("s_out", [NB * P, K], f32, kind="ExternalOutput")

    max_grp = max((CQB[q][b] for b in range(NB) for q in range(NQ)),
                  default=1)
    gorder = [(q, b) for q in range(NQ) for b in range(NB) if CQB[q][b] > 0]

    # one-hot load batches in consumption (= emission) order
    batches = []          # (cstart, cend); group -> (batch idx, col offset)
    group_batch = {}
    for i in range(0, len(gorder), OH_BATCH):
        gs = gorder[i:i + OH_BATCH]
        cstart = ohbase[gs[0][0] * NB + gs[0][1]]
        cend = ohbase[gs[-1][0] * NB + gs[-1][1]] + CQB[gs[-1][0]][gs[-1][1]]
        bi = len(batches)
        batches.append((cstart, cend))
        for (q, b) in gs:
            group_batch[(q, b)] = (bi, ohbase[q * NB + b] - cstart)
    max_batch_nch = max((ce - cs for cs, ce in batches), default=1)

    with tile.TileContext(nc) as tc:
        with (
            tc.tile_pool(name="big", bufs=8) as bigp,
            tc.tile_pool(name="gtp", bufs=RING) as gtp,
            tc.tile_pool(name="ohp", bufs=3) as ohp,
            tc.tile_pool(name="persist", bufs=1) as persist,
            tc.tile_pool(name="stage", bufs=3) as stagep,
            tc.tile_pool(name="tmp", bufs=3) as tmpp,
            tc.tile_pool(name="small", bufs=4) as smallp,
            tc.tile_pool(name="svp", bufs=1) as svp,
            tc.tile_pool(name="pa", bufs=4, space="PSUM") as pa,
            tc.tile_pool(name="pb", bufs=3, space="PSUM") as pb,
            tc.tile_pool(name="red", bufs=1, space="PSUM") as redp,
            tc.tile_pool(name="dram", bufs=1, space="DRAM") as dramp,
        ):
            # ---- constants / resident tensors
            ident = persist.tile([P, P], f32)
            make_identity(nc, ident[:])
            ident_b = persist.tile([P, P], bf16, tag="identb")
            nc.vector.tensor_copy(ident_b[:], ident[:])
            w1t_t = persist.tile([P, FT, D], bf16)
            for t in range(FT):
                fr = min(P, F - t * P)
                nc.sync.dma_start(w1t_t[:fr, t, :], w1t[t * P:t * P + fr, :])
            wtt_t = persist.tile([P, DT, K], bf16)
            for t in range(DT):
                nc.sync.dma_start(wtt_t[:, t, :], wtt[t * P:(t + 1) * P, :])

            ln_la = persist.tile([P, 1], f32, tag="lnla")
            nc.vector.memset(ln_la[:], LN_SELU_LA)
            la_c = persist.tile([P, 1], f32, tag="lac")
            nc.vector.memset(la_c[:], SELU_LA)
            ones_b = persist.tile([P, 1], bf16, tag="onesb")
            nc.vector.memset(ones_b[:], 1.0)
            ones_f = persist.tile([P, 1], f32, tag="onesf")
            nc.vector.memset(ones_f[:], 1.0)
            # masked ones: 1.0 for partitions < last-block rows, else 0
            # (partition-offset memset is rejected by the verifier, so
            # build it as a row-slice reduction of the identity matrix)
            last_rows = NSH - (NB - 1) * P
            ones_mf = persist.tile([P, 1], f32, tag="onesmf")
            nc.vector.reduce_sum(ones_mf[:], ident[:, 0:last_rows],
                                 axis=mybir.AxisListType.X)
            ones_m = persist.tile([P, 1], bf16, tag="onesm")
            nc.vector.tensor_copy(ones_m[:], ones_mf[:])

            accs = persist.tile([P, VLEN], f32, tag="accs")
            nc.vector.memset(accs[:], 0.0)
            # fp8 spill: the gathered table is already fp8 (6% per-element),
            # so an fp8 partial-sum spill adds comparable, uncorrelated
            # noise; buys 25KB/partition for deeper pipeline rings
            acc_blocks = persist.tile([P, NB, W2], fp8, tag="accb")

            red = redp.tile([P, DT], f32, tag="red")

            cc_in = [dramp.tile([rows_q[q], W2], fp8, name=f"cc_in{q}")
                     for q in range(NQ)]
            cc_out = [dramp.tile([NC * rows_q[q], W2], fp8,
                                 addr_space="Shared", name=f"cc_out{q}")
                      for q in range(NQ)]

            # ================= phase A: h1|h2 = [X|Xa] @ W1^T =============
            # one pass per AG chunk; each pass's rows AllGather as soon as
            # the pass's writes complete, so SpMM gathers on table 0 can
            # overlap the later AG chunks.  xx loads live on the scalar
            # HWDGE ring and st writes on the sync ring: sharing one ring
            # queues the st writes behind megabytes of feature loads and
            # delays the AG triggers by ~50us.
            idx_t = []
            for q in range(NQ):
                b_lo, b_hi = PB[q], PB[q + 1]
                c0 = b_lo * P
                c1 = min(b_hi * P, NSH)
                W = c1 - c0
                xt_tiles = []
                for which, src in enumerate((xt, xta)):
                    tl = []
                    for t in range(FT):
                        fr = min(P, F - t * P)
                        xx = bigp.tile([P, W], fp8, tag="big",
                                       name=f"xx{q}_{which}_{t}")
                        nc.scalar.dma_start(xx[:fr, :],
                                            src[t * P:t * P + fr, c0:c1])
                        tl.append(xx)
                    xt_tiles.append(tl)
                for b in range(b_lo, b_hi):
                    rows = min(P, NSH - b * P)
                    off = b * P - c0
                    ptw = pb.tile([P, W2], f32, space="PSUM", tag="pb")
                    for which in range(2):
                        for t in range(FT):
                            fr = min(P, F - t * P)
                            nc.tensor.matmul(
                                ptw[:rows, which * D:(which + 1) * D],
                                lhsT=xt_tiles[which][t][:fr, off:off + rows],
                                rhs=w1t_t[:fr, t, :],
                                start=(t == 0), stop=(t == FT - 1),
                            )
                    st = stagep.tile([P, W2], fp8, tag="stage")
                    nc.vector.tensor_copy(st[:rows, :], ptw[:rows, :])
                    nc.sync.dma_start(cc_in[q][b * P - c0:b * P - c0 + rows, :],
                                      st[:rows, :])
                nc.gpsimd.collective_compute(
                    "AllGather", mybir.AluOpType.bypass,
                    replica_groups=[list(range(NC))],
                    ins=[cc_in[q][:]], outs=[cc_out[q][:]],
                )
                # gather metadata for table q: emitted after pass q's
                # cc_in writes so it never delays the AG trigger, but
                # early enough to be resident before its gathers start
                if q == 0:
                    gcnt_t = persist.tile([1, NQ * NB], i32, tag="gcnt")
                    nc.sync.dma_start(gcnt_t[:], gcnt_d[:])
                it = persist.tile([P, Lq[q] // 16], i16, tag=f"idx{q}")
                nc.sync.dma_start(it[:], idx_d[q][:])
                idx_t.append(it)

            # ================= gather stream (q-major emission) ===========
            gtile = {}
            prev_inst = None
            gq = 0
            cnt_reg = nc.gpsimd.alloc_register("gcnt_reg")
            for er, (q, b) in enumerate(gorder):
                sc = strm_base[q][b]
                n = CQB[q][b]
                gt = gtp.tile([P, max_grp, W2], fp8, tag="gt",
                              name=f"gt_{q}_{b}")
                if er < RING and n < max_grp:
                    # first ring pass leaves slices >= n uninitialized;
                    # zero them so later pad lanes never read NaN bits
                    nc.vector.memset(gt[:, n:max_grp, :], 0.0)
                ld = nc.gpsimd.reg_load(
                    cnt_reg, gcnt_t[0:1, q * NB + b:q * NB + b + 1])
                if prev_inst is not None:
                    tile.add_dep_helper(ld.ins, prev_inst, sync=False,
                                        reason="gather issue order")
                gi = nc.gpsimd.dma_gather(
                    gt[:, 0:n, :], cc_out[q][:],
                    idx_t[q][:, sc * 8:(sc + n) * 8],
                    num_idxs=n * P, num_idxs_reg=cnt_reg, elem_size=W2,
                    single_packet=False,
                    queue_num=gq % 4,
                )
                gq += 1
                tile.add_dep_helper(gi.ins, ld.ins, sync=False,
                                    reason="count reg load order")
                prev_inst = gi.ins
                gtile[(q, b)] = gt

            # ================= SpMM consumption + epilogues ===============
            oh_tiles = {}

            def get_oh(bi):
                if bi not in oh_tiles:
                    cs, ce = batches[bi]
                    oht = ohp.tile([P, max_batch_nch, P], fp8, tag="oh",
                                   name=f"oh_{bi}")
                    nc.scalar.dma_start(oht[:, 0:ce - cs, :],
                                        oh[:, cs:ce, :])
                    oh_tiles[bi] = oht
                return oh_tiles[bi]

            # issue the first few oh batch loads up front
            for bi in range(min(2, len(batches))):
                get_oh(bi)

            for q in range(NQ):
                for b in range(NB):
                    n = CQB[q][b]
                    rows = min(P, NSH - b * P)
                    accb = acc_blocks[:, b, :]
                    if n == 0:
                        if q == 0:
                            nc.vector.memset(accb, 0.0)
                        if q < NQ - 1:
                            continue
                    pt = None
                    if n > 0:
                        bi, coff = group_batch[(q, b)]
                        oht = get_oh(bi)
                        if bi + 1 < len(batches):
                            get_oh(bi + 1)
                        gt = gtile[(q, b)]
                        pt = pa.tile([P, W2], f32, space="PSUM", tag="pa")
                        for j in range(n):
                            nc.tensor.matmul(
                                pt[:], lhsT=oht[:, coff + j, :],
                                rhs=gt[:, j, :],
                                start=(j == 0), stop=(j == n - 1))
                    if q == 0:
                        nc.vector.tensor_copy(accb, pt[:])
                        continue
                    if q < NQ - 1:
                        nc.vector.tensor_tensor(accb, accb, pt[:],
                                                mybir.AluOpType.add)
                        continue

                    # ---- q == NQ-1: close the block ----
                    tot = tmpp.tile([P, W2], f32, tag="tot")
                    if pt is not None:
                        nc.vector.tensor_tensor(tot[:], pt[:], accb,
                                                mybir.AluOpType.add)
                    else:
                        nc.vector.tensor_copy(tot[:], accb)

                    # selu over the combined [128, 512] row:
                    # e2 = la*exp(x); e3 = relu(la - e2); r = relu(l*x)
                    # selu = r - e3   (bf16 out: the gathered table is fp8,
                    # so bf16 rounding here is noise)
                    e2 = tmpp.tile([P, W2], f32, tag="e2")
                    nc.scalar.activation(e2[:], tot[:],
                                         mybir.ActivationFunctionType.Exp,
                                         bias=ln_la[:])
                    e3 = tmpp.tile([P, W2], f32, tag="e3")
                    nc.scalar.activation(e3[:], e2[:],
                                         mybir.ActivationFunctionType.Relu,
                                         bias=la_c[:], scale=-1.0)
                    slu = tmpp.tile([P, W2], bf16, tag="slu")
                    nc.scalar.activation(slu[:], tot[:],
                                         mybir.ActivationFunctionType.Relu,
                                         scale=SELU_L)
                    nc.vector.tensor_tensor(slu[:], slu[:], e3[:],
                                            mybir.AluOpType.subtract)
                    g1 = slu[:, 0:D]
                    aug = slu[:, D:W2]

                    # con-loss partials
                    nc.vector.tensor_tensor(accs[:, 0:D], accs[:, 0:D], aug,
                                            mybir.AluOpType.add)
                    pr = tmpp.tile([P, D], bf16, tag="pr")
                    nc.vector.tensor_tensor(pr[:], aug, g1,
                                            mybir.AluOpType.mult)
                    nc.vector.tensor_tensor(accs[:, D:W2], accs[:, D:W2],
                                            pr[:], mybir.AluOpType.add)

                    # log-softmax stats: exp column-sums into persistent
                    # PSUM (no max subtraction needed; |g1| <~ 25)
                    eg = tmpp.tile([P, D], bf16, tag="eg")
                    nc.scalar.activation(eg[:], g1,
                                         mybir.ActivationFunctionType.Exp)
                    rvec = ones_m if rows < P else ones_b
                    for t in range(DT):
                        nc.tensor.matmul(red[:, t:t + 1],
                                         lhsT=eg[:, t * P:(t + 1) * P],
                                         rhs=rvec[:],
                                         start=(b == 0), stop=(b == NB - 1))

                    # S = softmax_K(g1 @ Wt^T)
                    g1T = tmpp.tile([P, D], bf16, tag="g1T")
                    for t in range(DT):
                        ptr = pb.tile([P, P], bf16, space="PSUM", tag="pb")
                        nc.tensor.transpose(ptr[:], g1[:, t * P:(t + 1) * P],
                                            ident_b[:])
                        nc.vector.tensor_copy(g1T[:, t * P:(t + 1) * P],
                                              ptr[:])
                    pl = pb.tile([P, K], f32, space="PSUM", tag="pb")
                    for t in range(DT):
                        nc.tensor.matmul(pl[:],
                                         lhsT=g1T[:, t * P:(t + 1) * P],
                                         rhs=wtt_t[:, t, :],
                                         start=(t == 0), stop=(t == DT - 1))
                    # no max subtraction: |logits| <~ 30, exp is f32-safe
                    ex = smallp.tile([P, K], f32, tag="ex")
                    sm = smallp.tile([P, 1], f32, tag="sm")
                    nc.scalar.activation(ex[:], pl[:],
                                         mybir.ActivationFunctionType.Exp,
                                         accum_out=sm[:])
                    rc = smallp.tile([P, 1], f32, tag="rc")
                    nc.vector.reciprocal(rc[:], sm[:])
                    sb = stagep.tile([P, K], f32, tag="sstage")
                    nc.scalar.mul(sb[:], ex[:], rc[:])
                    nc.sync.dma_start(s_out_d[b * P:(b + 1) * P, :], sb[:])

            # ---- final tiny reductions out
            pv = pa.tile([P, VLEN], f32, space="PSUM", tag="pa")
            nc.tensor.matmul(pv[0:1, :], lhsT=ones_f[:], rhs=accs[:],
                             start=True, stop=True)
            sv = svp.tile([1, VLEN], f32, tag="sv")
            nc.vector.tensor_copy(sv[:], pv[0:1, :])
            nc.sync.dma_start(stats_v_d[:], sv[:])
            se = svp.tile([P, DT], f32, tag="se")
            nc.vector.tensor_copy(se[:], red[:])
            nc.sync.dma_start(stats_e_d[:], se[:])

    nc.compile()
    return nc


# --------------------------------------------------------------------------
# host-side combine of per-core partials
# --------------------------------------------------------------------------

def combine(results, cfg, graph_row, graph_col, graph_vals):
    N, D, K = cfg["N"], cfg["D"], cfg["K"]
    NSH = N // NC
    DT = D // P
    E = float(graph_row.shape[0])

    colsum_aug = np.zeros(D)
    dot = 0.0
    expsum = np.zeros(D)
    S_full = np.zeros((N, K))
    for c in range(NC):
        sv = np.asarray(results[c]["stats_v"], dtype=np.float64).reshape(-1)
        colsum_aug += sv[0:D]
        dot += sv[D:2 * D].sum()
        se = np.asarray(results[c]["stats_e"], dtype=np.float64)  # [P, DT]
        expsum += se.T.reshape(-1)          # d = t*128 + r
        S_full[c * NSH:(c + 1) * NSH] = \
            np.asarray(results[c]["s_out"], dtype=np.float64)[:NSH]
    logZ = np.log(expsum)

    row = np.asarray(graph_row).astype(np.int64)
    col = np.asarray(graph_col).astype(np.int64)
    av = np.asarray(graph_vals).astype(np.float64)
    deg = np.bincount(col, weights=av, minlength=N).astype(np.float64)

    trace_gp = np.einsum('e,ek,ek->', av, S_full[row], S_full[col])
    nl = S_full.T @ deg
    clsz = S_full.sum(axis=0)

    spectral = -(trace_gp - (nl ** 2).sum() / (2.0 * E)) / (2.0 * E)
    cluster = (np.linalg.norm(clsz) / N * math.sqrt(K) - 1.0) * CLUSTER_REG
    con = -(dot - (logZ * colsum_aug).sum()) / D
    return spectral + cluster + CON_REG * con


# --------------------------------------------------------------------------
# entry point
# --------------------------------------------------------------------------

_BUILD_CACHE = {}


def kernel(features, aug_features, graph_row, graph_col, graph_vals, gn_vals,
           lbl, dense_graph, W1, b1, Wt, bt, _cfg=None, _trace=False):
    cfg = _cfg or FULL
    in_maps, meta = prep(features, aug_features, graph_row, graph_col,
                         gn_vals, W1, Wt, cfg)
    key = tuple(sorted((k, str(v)) for k, v in meta.items()))
    if key not in _BUILD_CACHE:
        _BUILD_CACHE[key] = build(meta)
    nc = _BUILD_CACHE[key]
    res = bass_utils.run_bass_kernel_spmd(nc, in_maps, core_ids=list(range(NC)),
                                          trace=_trace)
    loss = combine(res.results, cfg, graph_row, graph_col, graph_vals)
    out = np.array(loss, dtype=np.float32)
    if _trace:
        return out, res
    return out
